# revision 21
# baseline (speedup 1.0000x reference)
"""T5-style encoder self-attention (dense_transformer) on 8 Trainium2 NeuronCores.

Problem (full shapes): hidden [2,2048,2048], Wq/Wk/Wv/Wo [2048,2048],
rel_emb [32,32] (bidirectional T5 relative-position bias), mask [2,1,1,2048].

Sharding: data-parallel over batch (2) x tensor-parallel over heads (4 groups
of 8 heads) = 8 cores, Megatron-style. Each core computes a partial output
[2048,2048] (fp16) for its batch; the host sums 4 partials per batch in fp32.

Per-core kernel design (bf16 operands, fp32 PSUM accumulation), organized as
ONE continuous PE instruction stream so the tensor engine never idles (TRN2
p-state: any PE gap drops the clock 2.4->1.2 GHz and costs ~3us of half-rate
execution afterwards):
  - phase A: fused projection chunks. Each 512-column chunk of x^T is
    streamed ONCE and feeds pair-0 Q^T/K^T (lhsT=W slices) AND V for all 8
    heads (lhsT=x^T 128-col slices, rhs=Wv) -- 6 matmuls per xt tile.
  - attention slots (pair, q-chunk): per k-tile iteration the PE executes
    [scores(kt+2) | 2 feeder matmuls | PV(kt)] while ACT runs exp(kt) and
    DVE runs the Toeplitz bias multiply (both heads in ONE 3D-AP tensor op).
    Feeder matmuls are the NEXT pair's Q/K projection chunks (and, during
    pair-3 slots, output-projection chunks), pumped from generators at 1-2
    steps/iter.  PSUM: scores 2-deep (4 banks) + ctx accumulator (2) +
    feeder accumulator (2) = all 8 banks.
  - Q^T is stored with s REVERSED so the relative-position bias is a
    positive-shear Toeplitz; exp(bias) diagonals are HOST-precomputed from
    rel_emb (structural bucket pattern) and DMA'd with sheared APs.
  - softmax without max-subtraction (scores are O(1)); V_aug carries a ones
    block so PSUM rows replicate the denominator for free.  Even heads use
    [V|ones], odd heads [ones|V] so ctx rows land on partitions 0:64 / 64:128
    and every psum->sbuf copy is partition-aligned (runs on DVE).
  - normalization is per-(pair,qc), deferred via a DRAM bounce (reciprocal +
    stride-0 broadcast DMA), fully overlapped with later slots.
  - output projection is chunked (2 x 512-wide psum halves, contraction over
    the 4 head-pairs) and interleaved into pair-3 slots as soon as the
    corresponding q-range is normalized; the fp32->fp16 drain copies run on
    DVE and partial outputs are summed on the host.
"""

import collections
import math
import sys

for _p in ("/opt/trn_rl_repo",):
    if _p not in sys.path:
        sys.path.insert(0, _p)

import numpy as np

import concourse.bass as bass
import concourse.mybir as mybir
import concourse.tile as tile
from concourse import bacc
from concourse.bass_utils import run_bass_kernel_spmd

DT = mybir.dt
AF = mybir.ActivationFunctionType
OP = mybir.AluOpType

# ---- problem constants (hardcoded per contract) ----
B, S, D = 2, 2048, 2048
N_HEADS, D_KV = 32, 64
NUM_BUCKETS, MAX_DISTANCE = 32, 128
NCORES = 8
HL = 8            # heads per core
P = 128
SC = 512          # free-dim chunk
NKT = S // P      # 16 k-tiles
NQC = S // SC     # 4 q-chunks
NDT = D // P      # 16 D-tiles
NMT = (HL * D_KV) // P   # 4 hd m-tiles per core
W_U = 3968        # toeplitz tile width: max(kt*P + qc*SC) + SC
NDIAG = 4096      # ediag row stride (4095 used)


def _rel_bucket_host(d):
    """Exact numpy replica of reference._relative_position_bucket (fp32 math,
    int32 truncation) for bidirectional buckets. d = k - q (int array)."""
    num_buckets = NUM_BUCKETS // 2          # 16
    max_exact = num_buckets // 2            # 8
    rel = np.asarray(d, dtype=np.int64)
    buckets = (rel > 0).astype(np.int32) * num_buckets
    arel = np.abs(rel)
    is_small = arel < max_exact
    rp_safe = np.maximum(arel, 1).astype(np.float32)
    log_ratio = np.log(rp_safe / np.float32(max_exact)).astype(np.float32)
    scale = np.float32(math.log(MAX_DISTANCE / max_exact))
    rp_large = max_exact + (log_ratio / scale * np.float32(num_buckets - max_exact)).astype(np.int32)
    rp_large = np.minimum(rp_large, num_buckets - 1)
    buckets = buckets + np.where(is_small, arel.astype(np.int32), rp_large)
    return buckets.astype(np.int32)


_BUCKETS = _rel_bucket_host(np.arange(NDIAG) - (S - 1))  # diag index -> bucket


def _rev_ap(base, jg0):
    """Reversed-q write AP into a [*, S]-shaped row range (un-reverse)."""
    return bass.AP(
        tensor=base.tensor,
        offset=base.offset + (S - 1 - jg0),
        ap=[list(base.ap[0]), [-1, SC]],
    )


def _build():
    nc = bacc.Bacc(None, name="attn_tp2")

    xt = nc.declare_dram_parameter("xt", [D, S], DT.bfloat16, isOutput=False)
    wq = nc.declare_dram_parameter("wq", [D, HL * D_KV], DT.bfloat16, isOutput=False)
    wk = nc.declare_dram_parameter("wk", [D, HL * D_KV], DT.bfloat16, isOutput=False)
    wv = nc.declare_dram_parameter("wv", [D, HL * D_KV], DT.bfloat16, isOutput=False)
    wo = nc.declare_dram_parameter("wo", [HL * D_KV, D], DT.bfloat16, isOutput=False)
    mask = nc.declare_dram_parameter("mask", [S], DT.float32, isOutput=False)
    ediag = nc.declare_dram_parameter("ediag", [HL, NDIAG], DT.bfloat16, isOutput=False)
    out = nc.declare_dram_parameter("out", [S, D], DT.float16, isOutput=True)

    with tile.TileContext(nc) as tc:
        with (
            tc.tile_pool(name="res", bufs=1) as res,
            tc.tile_pool(name="xtp", bufs=5) as xtp,
            tc.tile_pool(name="stage", bufs=2) as stage,
            tc.tile_pool(name="upool", bufs=2) as upool,
            tc.tile_pool(name="pexp", bufs=3) as pexpp,
            tc.tile_pool(name="outp", bufs=2) as outp,
            tc.tile_pool(name="psum", bufs=1, space="PSUM") as psum,
            tc.tile_pool(name="dram", bufs=1, space="DRAM") as dramp,
        ):
            # ---------- resident constants / weights ----------
            # hosts pre-transposes weights/mask to the SBUF layout so every
            # DMA below is a contiguous per-partition burst
            mask_sb = res.tile([P, NKT], DT.float32, tag="mask")
            nc.sync.dma_start(mask_sb[:], mask.ap().rearrange("(p kt) -> p kt", p=P))

            wq_sb = res.tile([P, NDT, HL * D_KV], DT.bfloat16, tag="wq")
            wk_sb = res.tile([P, NDT, HL * D_KV], DT.bfloat16, tag="wk")
            wv_sb = res.tile([P, NDT, HL * D_KV], DT.bfloat16, tag="wv")
            # pair-0 weight columns first so phase A's first matmuls start
            # without waiting for the full 6MB of weights
            wq_r = wq.ap().rearrange("(p kt) h -> p kt h", p=P)
            wk_r = wk.ap().rearrange("(p kt) h -> p kt h", p=P)
            nc.sync.dma_start(wq_sb[:, :, 0:P], wq_r[:, :, 0:P])
            nc.sync.dma_start(wk_sb[:, :, 0:P], wk_r[:, :, 0:P])
            # wv split per-kd so V matmuls start as soon as their block lands
            wv_r = wv.ap().rearrange("(p kt) h -> p kt h", p=P)
            for kd in range(NDT):
                nc.sync.dma_start(wv_sb[:, kd, :], wv_r[:, kd, :])

            # preload the ACT exp table early (off the critical path)
            warm = stage.tile([1, 1], DT.float32, tag="warm")
            nc.scalar.activation(out=warm[:], in_=mask_sb[0:1, 0:1], func=AF.Exp)

            wo_sb = res.tile([P, NMT, D], DT.bfloat16, tag="wo")

            # persistent activations
            qt_sb = res.tile([P, NMT, S], DT.bfloat16, tag="qt")   # q REVERSED
            kt_sb = res.tile([P, NMT, S], DT.bfloat16, tag="kt")
            # V_aug: even heads [V | ones], odd heads [ones | V] so ctx rows
            # land on the matching ctxt partitions.
            vaug = res.tile([P, NKT, HL, 2 * D_KV], DT.bfloat16, tag="vaug")
            ctxt = res.tile([P, NMT, S], DT.bfloat16, tag="ctxt")
            nc.vector.memset(vaug[:], 1.0)

            den_dram = dramp.tile([HL * NQC, SC], DT.float32)
            rcp_dram = dramp.tile([HL * NQC, SC], DT.float32)

            # ---------- helpers ----------
            def load_u(pr):
                """Merged Toeplitz exp-bias tile for pair pr: [P, 2, W_U]."""
                u = upool.tile([P, 2, W_U], DT.bfloat16, tag="u", name=f"u{pr}")
                for i in (0, 1):
                    shear = bass.AP(
                        tensor=ediag.ap().tensor,
                        offset=ediag.ap().offset + (2 * pr + i) * NDIAG,
                        ap=[[1, P], [1, W_U]],
                    )
                    nc.sync.dma_start(u[:, i, :], shear)
                return u

            def xt_dma(tag_name, kd, nq):
                t = xtp.tile([P, SC], DT.bfloat16, tag="xt", name=f"{tag_name}_{kd}")
                nc.sync.dma_start(t[:], xt[kd * P:(kd + 1) * P, nq * SC:(nq + 1) * SC])
                return t

            def drain_qk(pr, nq, qk_ps):
                """Write reversed Q^T and K^T chunks from psum (DVE)."""
                dst = qt_sb[:, pr, :]
                nc.vector.tensor_copy(_rev_ap(dst, nq * SC), qk_ps[:, 0:SC])
                nc.vector.tensor_copy(
                    kt_sb[:, pr, nq * SC:(nq + 1) * SC], qk_ps[:, SC:2 * SC]
                )

            def drain_v(nq, st_pair, v_ps):
                """Scatter V into vaug with per-parity column offsets (DVE)."""
                for j in (0, 1):
                    st = 2 * st_pair + j
                    kt_glob = nq * 4 + st
                    src = v_ps[:, j * SC:(j + 1) * SC].rearrange(
                        "p (h d) -> p h d", d=D_KV
                    )
                    # even heads -> cols 0:64, odd heads -> cols 64:128
                    nc.vector.tensor_copy(
                        vaug[:, kt_glob, 0::2, 0:D_KV], src[:, 0::2, :]
                    )
                    nc.vector.tensor_copy(
                        vaug[:, kt_glob, 1::2, D_KV:2 * D_KV], src[:, 1::2, :]
                    )

            # ---------- phase A: fused pair-0 QK + all-head V ----------
            def phase_a_chunk(nq):
                qk_ps = psum.tile([P, 2 * SC], DT.float32, tag="sc", bufs=2,
                                  name=f"Aqk{nq}")
                v01 = psum.tile([P, 2 * SC], DT.float32, tag="cx", bufs=1,
                                name=f"Av01_{nq}")
                v23 = psum.tile([P, 2 * SC], DT.float32, tag="aux", bufs=1,
                                name=f"Av23_{nq}")
                v_ps = [v01[:, 0:SC], v01[:, SC:2 * SC],
                        v23[:, 0:SC], v23[:, SC:2 * SC]]
                tiles = {kd: xt_dma(f"Ax{nq}", kd, nq) for kd in range(3)}
                for kd in range(NDT):
                    xt_t = tiles.pop(kd)
                    nc.tensor.matmul(
                        qk_ps[:, 0:SC], wq_sb[:, kd, 0:P], xt_t[:],
                        start=(kd == 0), stop=(kd == NDT - 1),
                    )
                    nc.tensor.matmul(
                        qk_ps[:, SC:2 * SC], wk_sb[:, kd, 0:P], xt_t[:],
                        start=(kd == 0), stop=(kd == NDT - 1),
                    )
                    for st in range(4):
                        nc.tensor.matmul(
                            v_ps[st], xt_t[:, st * P:(st + 1) * P],
                            wv_sb[:, kd, :],
                            start=(kd == 0), stop=(kd == NDT - 1),
                        )
                    if kd + 3 < NDT:
                        tiles[kd + 3] = xt_dma(f"Ax{nq}", kd + 3, nq)
                drain_qk(0, nq, qk_ps)
                drain_v(nq, 0, v01)
                drain_v(nq, 1, v23)

            # bulk DMAs staged between chunks, earliest-needed first, so
            # they never sit ahead of the next chunk's xt stream
            phase_a_chunk(0)
            u_cur = load_u(0)
            phase_a_chunk(1)
            nc.sync.dma_start(wq_sb[:, :, P:2 * P], wq_r[:, :, P:2 * P])
            nc.sync.dma_start(wk_sb[:, :, P:2 * P], wk_r[:, :, P:2 * P])
            phase_a_chunk(2)
            nc.sync.dma_start(wq_sb[:, :, 2 * P:HL * D_KV], wq_r[:, :, 2 * P:HL * D_KV])
            nc.sync.dma_start(wk_sb[:, :, 2 * P:HL * D_KV], wk_r[:, :, 2 * P:HL * D_KV])
            phase_a_chunk(3)
            nc.sync.dma_start(wo_sb[:], wo.ap().rearrange("(p mt) d -> p mt d", p=P))

            # ---------- feeder generators ----------
            def qk_chunk_gen(pr, nq):
                qk_ps = psum.tile([P, 2 * SC], DT.float32, tag="aux", bufs=1,
                                  name=f"qk{pr}_{nq}")
                tiles = {kd: xt_dma(f"x{pr}_{nq}", kd, nq) for kd in range(3)}
                for kd in range(NDT):
                    xt_t = tiles.pop(kd)
                    nc.tensor.matmul(
                        qk_ps[:, 0:SC], wq_sb[:, kd, pr * P:(pr + 1) * P],
                        xt_t[:], start=(kd == 0), stop=(kd == NDT - 1),
                    )
                    nc.tensor.matmul(
                        qk_ps[:, SC:2 * SC], wk_sb[:, kd, pr * P:(pr + 1) * P],
                        xt_t[:], start=(kd == 0), stop=(kd == NDT - 1),
                    )
                    if kd + 3 < NDT:
                        tiles[kd + 3] = xt_dma(f"x{pr}_{nq}", kd + 3, nq)
                    if kd < NDT - 1:
                        yield
                drain_qk(pr, nq, qk_ps)
                yield

            def out_chunk_gen(st, ndp, tag="aux", tag_bufs=1):
                """Output projection rows st*128.. for d-cols [ndp*1024, +1024)."""
                o_ps = psum.tile([P, 2 * SC], DT.float32, tag=tag, bufs=tag_bufs,
                                 name=f"o{st}_{ndp}")
                for m in range(NMT):
                    nc.tensor.matmul(
                        o_ps[:, 0:SC], ctxt[:, m, st * P:(st + 1) * P],
                        wo_sb[:, m, (2 * ndp) * SC:(2 * ndp + 1) * SC],
                        start=(m == 0), stop=(m == NMT - 1),
                    )
                    nc.tensor.matmul(
                        o_ps[:, SC:2 * SC], ctxt[:, m, st * P:(st + 1) * P],
                        wo_sb[:, m, (2 * ndp + 1) * SC:(2 * ndp + 2) * SC],
                        start=(m == 0), stop=(m == NMT - 1),
                    )
                    if m < NMT - 1:
                        yield
                o_t = outp.tile([P, 2 * SC], DT.float16, tag="out",
                                name=f"ot{st}_{ndp}")
                nc.vector.tensor_copy(o_t[:], o_ps[:])
                nc.sync.dma_start(
                    out[st * P:(st + 1) * P,
                        (2 * ndp) * SC:(2 * ndp + 2) * SC], o_t[:]
                )
                yield

            feeders = collections.deque()
            # next-pair QK chunks, ordered by when their outputs are consumed
            for pr in (1, 2, 3):
                for nq in (0, 3, 2, 1):
                    feeders.append(qk_chunk_gen(pr, nq))

            def pump():
                while feeders:
                    try:
                        next(feeders[0])
                        return True
                    except StopIteration:
                        feeders.popleft()
                return False

            # ---------- attention slots ----------
            def emit_scores(pr, qc, kt, t):
                s01 = psum.tile([P, 2, SC], DT.float32, tag="sc", bufs=2,
                                name=f"s{t}")
                jg0 = qc * SC
                nc.tensor.matmul(
                    s01[:, 0, :], kt_sb[0:64, pr, kt * P:(kt + 1) * P],
                    qt_sb[0:64, pr, jg0:jg0 + SC],
                    start=True, stop=True, tile_position=(0, 0),
                )
                nc.tensor.matmul(
                    s01[:, 1, :], kt_sb[64:128, pr, kt * P:(kt + 1) * P],
                    qt_sb[64:128, pr, jg0:jg0 + SC],
                    start=True, stop=True, tile_position=(64, 0),
                )
                return s01

            def norm_qc(pr, qc):
                """Deferred softmax division for (pair, qc); overlaps later slots."""
                den_sb = stage.tile([2, SC], DT.float32, tag="den2",
                                    name=f"dq{pr}_{qc}")
                rows = [2 * pr * NQC + qc, (2 * pr + 1) * NQC + qc]
                for r, row in enumerate(rows):
                    nc.sync.dma_start(den_sb[r:r + 1, :], den_dram[row, :])
                rcp2 = stage.tile([2, SC], DT.float32, tag="rcp2",
                                  name=f"rq{pr}_{qc}")
                nc.vector.reciprocal_approx_fast(rcp2[:], den_sb[:])
                for r, row in enumerate(rows):
                    nc.sync.dma_start(rcp_dram[row, :], rcp2[r:r + 1, :])
                q0t = S - (qc + 1) * SC
                for r in range(2):
                    hh = 2 * pr + r
                    off = r * 64
                    rb = stage.tile([P, SC], DT.float32, tag="rb",
                                    name=f"rb{hh}_{qc}")
                    bcast = bass.AP(
                        tensor=rcp_dram.tensor,
                        offset=rcp_dram.offset + (hh * NQC + qc) * SC,
                        ap=[[0, D_KV], [1, SC]],
                    )
                    nc.sync.dma_start(rb[off:off + D_KV, :], bcast)
                    cslc = ctxt[off:off + 64, pr, q0t:q0t + SC]
                    nc.gpsimd.tensor_tensor(
                        cslc, cslc, rb[off:off + D_KV, :], OP.mult
                    )

            NTOK = 16 * NKT  # 16 slots x 16 k-tiles
            slots = [(pr, qc) for pr in range(NMT) for qc in range(NQC)]

            def decode(t):
                si, kt = divmod(t, NKT)
                return slots[si][0], slots[si][1], kt

            u_next = None
            pend = {}
            pend[0] = emit_scores(*decode(0), 0)
            pend[1] = emit_scores(*decode(1), 1)

            cx01 = None
            for t in range(NTOK):
                pr, qc, kt = decode(t)
                jg0 = qc * SC
                if kt == 0:
                    if qc == 0 and pr + 1 < NMT:
                        u_next = load_u(pr + 1)
                    cx01 = psum.tile([P, 2 * SC], DT.float32, tag="cx", bufs=1,
                                     name=f"cx{pr}_{qc}")
                s01 = pend.pop(t)
                # ACT: exp(scores/8 + mask_k), psum -> sbuf bf16, both heads
                px = pexpp.tile([P, 2, SC], DT.bfloat16, tag="pexp",
                                name=f"px{t}")
                nc.scalar.activation(
                    out=px[:], in_=s01[:], func=AF.Exp,
                    bias=mask_sb[:, kt:kt + 1], scale=1.0 / math.sqrt(D_KV),
                )
                # DVE: multiply by exp(bias) Toeplitz, both heads in one op
                j0 = kt * P + jg0
                nc.vector.tensor_tensor(
                    px[:], px[:], u_cur[:, :, j0:j0 + SC], OP.mult
                )
                # PE: scores two iterations ahead
                if t + 2 < NTOK:
                    pend[t + 2] = emit_scores(*decode(t + 2), t + 2)
                # feeder matmuls keep the PE saturated past ACT's rate (and
                # give the px chain latency cover before PV needs it)
                pump()
                if kt < 2:
                    pump()
                # PE: PV for both heads (even: [V|ones], odd: [ones|V])
                nc.tensor.matmul(
                    cx01[:, 0:SC], vaug[:, kt, 2 * pr, :], px[:, 0, :],
                    start=(kt == 0), stop=(kt == NKT - 1),
                )
                nc.tensor.matmul(
                    cx01[:, SC:2 * SC], vaug[:, kt, 2 * pr + 1, :], px[:, 1, :],
                    start=(kt == 0), stop=(kt == NKT - 1),
                )
                if kt == NKT - 1:
                    # drain slot: ctx rows + denominator rows (all DVE,
                    # partition-aligned), then deferred normalize
                    cx0, cx1 = cx01[:, 0:SC], cx01[:, SC:2 * SC]
                    # h0 ctx copy on ACT, h1 on DVE: both finish ~0.7us after
                    # PV(15) so the cx psum slot frees before the next slot's
                    # PV(0) arrives
                    dst0 = ctxt[0:64, pr, :]
                    nc.scalar.copy(_rev_ap(dst0, jg0), cx0[0:D_KV, :])
                    dst1 = ctxt[64:128, pr, :]
                    nc.vector.tensor_copy(_rev_ap(dst1, jg0), cx1[64:128, :])
                    dn = stage.tile([P, SC], DT.float32, tag="dn",
                                    name=f"dn{pr}_{qc}")
                    for r, (src_row, dn_row) in enumerate(((64, 64), (0, 0))):
                        csrc = (cx0, cx1)[r][src_row:src_row + 1, :]
                        dslc = dn[dn_row:dn_row + 1, :]
                        drev = bass.AP(
                            tensor=dslc.tensor,
                            offset=dslc.offset + (SC - 1),
                            ap=[list(dslc.ap[0]), [-1, SC]],
                        )
                        nc.vector.tensor_copy(drev, csrc)
                        nc.sync.dma_start(
                            den_dram[(2 * pr + r) * NQC + qc, :],
                            dn[dn_row:dn_row + 1, :],
                        )
                    norm_qc(pr, qc)
                    if pr == NMT - 1:
                        # out-proj rows for this (now fully normalized) q-range.
                        # The last group runs in phase C where the attention
                        # psum tags are free: rotate tags so drains overlap.
                        last = qc == NQC - 1
                        tags = (("sc", 2), ("cx", 1), ("aux", 1))
                        st0 = (S - (qc + 1) * SC) // P
                        for i, (st, ndp) in enumerate(
                            (st, ndp)
                            for st in range(st0, st0 + SC // P)
                            for ndp in range(2)
                        ):
                            tg, tb = tags[i % 3] if last else ("aux", 1)
                            feeders.append(out_chunk_gen(st, ndp, tg, tb))
                    if qc == NQC - 1:
                        u_cur = u_next

            # ---------- phase C: remaining output projection ----------
            while pump():
                pass

    nc.finalize()
    return nc


_NC_CACHE = None


def _get_nc():
    global _NC_CACHE
    if _NC_CACHE is None:
        _NC_CACHE = _build()
    return _NC_CACHE


def _in_maps(hidden_states, attention_mask, Wq, Wk, Wv, Wo, rel_emb):
    import ml_dtypes
    bf16 = ml_dtypes.bfloat16
    # host-side structural gather: exp(rel bias) along score diagonals
    ediag_full = np.exp(rel_emb[_BUCKETS, :].astype(np.float64)).astype(bf16)
    maps = []
    for c in range(NCORES):
        b, g = c // 4, c % 4
        hlo, hhi = g * HL, (g + 1) * HL
        def _wlayout(w):  # [D, hd] -> [(p kt) h] SBUF-matched layout
            return np.ascontiguousarray(
                w.reshape(NDT, P, HL * D_KV).transpose(1, 0, 2).reshape(D, HL * D_KV)
            ).astype(bf16)
        wo_c = Wo[hlo * D_KV:hhi * D_KV, :]
        maps.append({
            "xt": np.ascontiguousarray(hidden_states[b].T).astype(bf16),
            "wq": _wlayout(Wq[:, hlo * D_KV:hhi * D_KV]),
            "wk": _wlayout(Wk[:, hlo * D_KV:hhi * D_KV]),
            "wv": _wlayout(Wv[:, hlo * D_KV:hhi * D_KV]),
            "wo": np.ascontiguousarray(
                wo_c.reshape(NMT, P, D).transpose(1, 0, 2).reshape(HL * D_KV, D)
            ).astype(bf16),
            "mask": np.ascontiguousarray(
                attention_mask[b, 0, 0, :].reshape(NKT, P).T.reshape(S)
            ).astype(np.float32),
            "ediag": np.ascontiguousarray(ediag_full[:, hlo:hhi].T),
        })
    return maps


def kernel(hidden_states, attention_mask, Wq, Wk, Wv, Wo, rel_emb, _trace=False,
           _trace_kwargs=None):
    hidden_states = np.asarray(hidden_states, dtype=np.float32)
    attention_mask = np.asarray(attention_mask, dtype=np.float32)
    Wq = np.asarray(Wq, dtype=np.float32)
    Wk = np.asarray(Wk, dtype=np.float32)
    Wv = np.asarray(Wv, dtype=np.float32)
    Wo = np.asarray(Wo, dtype=np.float32)
    rel_emb = np.asarray(rel_emb, dtype=np.float32)

    nc = _get_nc()
    maps = _in_maps(hidden_states, attention_mask, Wq, Wk, Wv, Wo, rel_emb)
    kw = dict(_trace_kwargs or {})
    res = run_bass_kernel_spmd(nc, maps, core_ids=list(range(NCORES)),
                               trace=_trace, **kw)
    kernel.last_results = res
    outp = np.empty((B, S, D), dtype=np.float32)
    for b in range(B):
        acc = np.asarray(res.results[4 * b]["out"], dtype=np.float32).copy()
        for g in range(1, 4):
            acc += np.asarray(res.results[4 * b + g]["out"], dtype=np.float32)
        outp[b] = acc
    return outp


# revision 22
# speedup vs baseline: 1.0003x; 1.0003x over previous
"""T5-style encoder self-attention (dense_transformer) on 8 Trainium2 NeuronCores.

Problem (full shapes): hidden [2,2048,2048], Wq/Wk/Wv/Wo [2048,2048],
rel_emb [32,32] (bidirectional T5 relative-position bias), mask [2,1,1,2048].

Sharding: data-parallel over batch (2) x tensor-parallel over heads (4 groups
of 8 heads) = 8 cores, Megatron-style. Each core computes a partial output
[2048,2048] (fp16) for its batch; the host sums 4 partials per batch in fp32.

Per-core kernel design (bf16 operands, fp32 PSUM accumulation), organized as
ONE continuous PE instruction stream so the tensor engine never idles (TRN2
p-state: any PE gap drops the clock 2.4->1.2 GHz and costs ~3us of half-rate
execution afterwards):
  - phase A: fused projection chunks. Each 512-column chunk of x^T is
    streamed ONCE and feeds pair-0 Q^T/K^T (lhsT=W slices) AND V for all 8
    heads (lhsT=x^T 128-col slices, rhs=Wv) -- 6 matmuls per xt tile.
  - attention slots (pair, q-chunk): per k-tile iteration the PE executes
    [scores(kt+2) | 2 feeder matmuls | PV(kt)] while ACT runs exp(kt) and
    DVE runs the Toeplitz bias multiply (both heads in ONE 3D-AP tensor op).
    Feeder matmuls are the NEXT pair's Q/K projection chunks (and, during
    pair-3 slots, output-projection chunks), pumped from generators at 1-2
    steps/iter.  PSUM: scores 2-deep (4 banks) + ctx accumulator (2) +
    feeder accumulator (2) = all 8 banks.
  - Q^T is stored with s REVERSED so the relative-position bias is a
    positive-shear Toeplitz; exp(bias) diagonals are HOST-precomputed from
    rel_emb (structural bucket pattern) and DMA'd with sheared APs.
  - softmax without max-subtraction (scores are O(1)); V_aug carries a ones
    block so PSUM rows replicate the denominator for free.  Even heads use
    [V|ones], odd heads [ones|V] so ctx rows land on partitions 0:64 / 64:128
    and every psum->sbuf copy is partition-aligned (runs on DVE).
  - normalization is per-(pair,qc), deferred via a DRAM bounce (reciprocal +
    stride-0 broadcast DMA), fully overlapped with later slots.
  - output projection is chunked (2 x 512-wide psum halves, contraction over
    the 4 head-pairs) and interleaved into pair-3 slots as soon as the
    corresponding q-range is normalized; the fp32->fp16 drain copies run on
    DVE and partial outputs are summed on the host.
"""

import collections
import math
import sys

for _p in ("/opt/trn_rl_repo",):
    if _p not in sys.path:
        sys.path.insert(0, _p)

import numpy as np

import concourse.bass as bass
import concourse.mybir as mybir
import concourse.tile as tile
from concourse import bacc
from concourse.bass_utils import run_bass_kernel_spmd

DT = mybir.dt
AF = mybir.ActivationFunctionType
OP = mybir.AluOpType

# ---- problem constants (hardcoded per contract) ----
B, S, D = 2, 2048, 2048
N_HEADS, D_KV = 32, 64
NUM_BUCKETS, MAX_DISTANCE = 32, 128
NCORES = 8
HL = 8            # heads per core
P = 128
SC = 512          # free-dim chunk
NKT = S // P      # 16 k-tiles
NQC = S // SC     # 4 q-chunks
NDT = D // P      # 16 D-tiles
NMT = (HL * D_KV) // P   # 4 hd m-tiles per core
W_U = 3968        # toeplitz tile width: max(kt*P + qc*SC) + SC
NDIAG = 4096      # ediag row stride (4095 used)


def _rel_bucket_host(d):
    """Exact numpy replica of reference._relative_position_bucket (fp32 math,
    int32 truncation) for bidirectional buckets. d = k - q (int array)."""
    num_buckets = NUM_BUCKETS // 2          # 16
    max_exact = num_buckets // 2            # 8
    rel = np.asarray(d, dtype=np.int64)
    buckets = (rel > 0).astype(np.int32) * num_buckets
    arel = np.abs(rel)
    is_small = arel < max_exact
    rp_safe = np.maximum(arel, 1).astype(np.float32)
    log_ratio = np.log(rp_safe / np.float32(max_exact)).astype(np.float32)
    scale = np.float32(math.log(MAX_DISTANCE / max_exact))
    rp_large = max_exact + (log_ratio / scale * np.float32(num_buckets - max_exact)).astype(np.int32)
    rp_large = np.minimum(rp_large, num_buckets - 1)
    buckets = buckets + np.where(is_small, arel.astype(np.int32), rp_large)
    return buckets.astype(np.int32)


_BUCKETS = _rel_bucket_host(np.arange(NDIAG) - (S - 1))  # diag index -> bucket


def _rev_ap(base, jg0):
    """Reversed-q write AP into a [*, S]-shaped row range (un-reverse)."""
    return bass.AP(
        tensor=base.tensor,
        offset=base.offset + (S - 1 - jg0),
        ap=[list(base.ap[0]), [-1, SC]],
    )


def _build():
    nc = bacc.Bacc(None, name="attn_tp2")

    xt = nc.declare_dram_parameter("xt", [D, S], DT.bfloat16, isOutput=False)
    wq = nc.declare_dram_parameter("wq", [P, NMT, NDT, P], DT.bfloat16, isOutput=False)
    wk = nc.declare_dram_parameter("wk", [P, NMT, NDT, P], DT.bfloat16, isOutput=False)
    wv = nc.declare_dram_parameter("wv", [D, HL * D_KV], DT.bfloat16, isOutput=False)
    wo = nc.declare_dram_parameter("wo", [HL * D_KV, D], DT.bfloat16, isOutput=False)
    mask = nc.declare_dram_parameter("mask", [S], DT.float32, isOutput=False)
    ediag = nc.declare_dram_parameter("ediag", [HL, NDIAG], DT.bfloat16, isOutput=False)
    out = nc.declare_dram_parameter("out", [S, D], DT.float16, isOutput=True)

    with tile.TileContext(nc) as tc:
        with (
            tc.tile_pool(name="res", bufs=1) as res,
            tc.tile_pool(name="xtp", bufs=6) as xtp,
            tc.tile_pool(name="stage", bufs=1) as stage,
            tc.tile_pool(name="upool", bufs=2) as upool,
            tc.tile_pool(name="pexp", bufs=3) as pexpp,
            tc.tile_pool(name="outp", bufs=2) as outp,
            tc.tile_pool(name="psum", bufs=1, space="PSUM") as psum,
            tc.tile_pool(name="dram", bufs=1, space="DRAM") as dramp,
        ):
            # ---------- resident constants / weights ----------
            # hosts pre-transposes weights/mask to the SBUF layout so every
            # DMA below is a contiguous per-partition burst
            mask_sb = res.tile([P, NKT], DT.float32, tag="mask")
            nc.sync.dma_start(mask_sb[:], mask.ap().rearrange("(p kt) -> p kt", p=P))

            # wq/wk are pair-major [p, pr, kd, c] so each pair's block is one
            # contiguous-burst DMA; pair 0 lands first
            wq_sb = res.tile([P, NMT, NDT, P], DT.bfloat16, tag="wq")
            wk_sb = res.tile([P, NMT, NDT, P], DT.bfloat16, tag="wk")
            wv_sb = res.tile([P, NDT, HL * D_KV], DT.bfloat16, tag="wv")
            nc.sync.dma_start(wq_sb[:, 0], wq[:, 0])
            nc.sync.dma_start(wk_sb[:, 0], wk[:, 0])
            # wv split per-kd so V matmuls start as soon as their block lands
            wv_r = wv.ap().rearrange("(p kt) h -> p kt h", p=P)
            for kd in range(NDT):
                nc.sync.dma_start(wv_sb[:, kd, :], wv_r[:, kd, :])

            # preload the ACT exp table early (off the critical path)
            warm = stage.tile([1, 1], DT.float32, tag="warm")
            nc.scalar.activation(out=warm[:], in_=mask_sb[0:1, 0:1], func=AF.Exp)

            wo_sb = res.tile([P, NMT, D], DT.bfloat16, tag="wo")

            # persistent activations
            qt_sb = res.tile([P, NMT, S], DT.bfloat16, tag="qt")   # q REVERSED
            kt_sb = res.tile([P, NMT, S], DT.bfloat16, tag="kt")
            # V_aug: even heads [V | ones], odd heads [ones | V] so ctx rows
            # land on the matching ctxt partitions.
            vaug = res.tile([P, NKT, HL, 2 * D_KV], DT.bfloat16, tag="vaug")
            ctxt = res.tile([P, NMT, S], DT.bfloat16, tag="ctxt")
            nc.vector.memset(vaug[:], 1.0)

            den_dram = dramp.tile([HL * NQC, SC], DT.float32)
            rcp_dram = dramp.tile([HL * NQC, SC], DT.float32)

            # ---------- helpers ----------
            def load_u(pr):
                """Merged Toeplitz exp-bias tile for pair pr: [P, 2, W_U]."""
                u = upool.tile([P, 2, W_U], DT.bfloat16, tag="u", name=f"u{pr}")
                for i in (0, 1):
                    shear = bass.AP(
                        tensor=ediag.ap().tensor,
                        offset=ediag.ap().offset + (2 * pr + i) * NDIAG,
                        ap=[[1, P], [1, W_U]],
                    )
                    nc.sync.dma_start(u[:, i, :], shear)
                return u

            def xt_dma(tag_name, kd, nq):
                t = xtp.tile([P, SC], DT.bfloat16, tag="xt", name=f"{tag_name}_{kd}")
                nc.sync.dma_start(t[:], xt[kd * P:(kd + 1) * P, nq * SC:(nq + 1) * SC])
                return t

            def drain_qk(pr, nq, qk_ps):
                """Write reversed Q^T and K^T chunks from psum (DVE)."""
                dst = qt_sb[:, pr, :]
                nc.vector.tensor_copy(_rev_ap(dst, nq * SC), qk_ps[:, 0:SC])
                nc.vector.tensor_copy(
                    kt_sb[:, pr, nq * SC:(nq + 1) * SC], qk_ps[:, SC:2 * SC]
                )

            def drain_v(nq, st_pair, v_ps):
                """Scatter V into vaug with per-parity column offsets (DVE)."""
                for j in (0, 1):
                    st = 2 * st_pair + j
                    kt_glob = nq * 4 + st
                    src = v_ps[:, j * SC:(j + 1) * SC].rearrange(
                        "p (h d) -> p h d", d=D_KV
                    )
                    # even heads -> cols 0:64, odd heads -> cols 64:128
                    nc.vector.tensor_copy(
                        vaug[:, kt_glob, 0::2, 0:D_KV], src[:, 0::2, :]
                    )
                    nc.vector.tensor_copy(
                        vaug[:, kt_glob, 1::2, D_KV:2 * D_KV], src[:, 1::2, :]
                    )

            # ---------- phase A: fused pair-0 QK + all-head V ----------
            def phase_a_chunk(nq, next_nq=None):
                qk_ps = psum.tile([P, 2 * SC], DT.float32, tag="sc", bufs=2,
                                  name=f"Aqk{nq}")
                v01 = psum.tile([P, 2 * SC], DT.float32, tag="cx", bufs=1,
                                name=f"Av01_{nq}")
                v23 = psum.tile([P, 2 * SC], DT.float32, tag="aux", bufs=1,
                                name=f"Av23_{nq}")
                v_ps = [v01[:, 0:SC], v01[:, SC:2 * SC],
                        v23[:, 0:SC], v23[:, SC:2 * SC]]
                tiles = dict(a_prefetch) if a_prefetch else {
                    kd: xt_dma(f"Ax{nq}", kd, nq) for kd in range(3)}
                a_prefetch.clear()
                for kd in range(NDT):
                    xt_t = tiles.pop(kd)
                    nc.tensor.matmul(
                        qk_ps[:, 0:SC], wq_sb[:, 0, kd, :], xt_t[:],
                        start=(kd == 0), stop=(kd == NDT - 1),
                    )
                    nc.tensor.matmul(
                        qk_ps[:, SC:2 * SC], wk_sb[:, 0, kd, :], xt_t[:],
                        start=(kd == 0), stop=(kd == NDT - 1),
                    )
                    for st in range(4):
                        nc.tensor.matmul(
                            v_ps[st], xt_t[:, st * P:(st + 1) * P],
                            wv_sb[:, kd, :],
                            start=(kd == 0), stop=(kd == NDT - 1),
                        )
                    if kd + 3 < NDT:
                        tiles[kd + 3] = xt_dma(f"Ax{nq}", kd + 3, nq)
                    elif next_nq is not None:
                        k2 = kd + 3 - NDT
                        a_prefetch[k2] = xt_dma(f"Ax{next_nq}", k2, next_nq)
                drain_qk(0, nq, qk_ps)
                drain_v(nq, 0, v01)
                drain_v(nq, 1, v23)

            # bulk DMAs staged between chunks, earliest-needed first, so
            # they never sit ahead of the next chunk's xt stream
            a_prefetch = {}
            phase_a_chunk(0, next_nq=1)
            u_cur = load_u(0)
            phase_a_chunk(1, next_nq=2)
            nc.sync.dma_start(wq_sb[:, 1], wq[:, 1])
            nc.sync.dma_start(wk_sb[:, 1], wk[:, 1])
            phase_a_chunk(2, next_nq=3)
            nc.sync.dma_start(wq_sb[:, 2:4], wq[:, 2:4])
            nc.sync.dma_start(wk_sb[:, 2:4], wk[:, 2:4])
            phase_a_chunk(3)
            nc.sync.dma_start(wo_sb[:], wo.ap().rearrange("(p mt) d -> p mt d", p=P))

            # ---------- feeder generators ----------
            def qk_chunk_gen(pr, nq):
                qk_ps = psum.tile([P, 2 * SC], DT.float32, tag="aux", bufs=1,
                                  name=f"qk{pr}_{nq}")
                tiles = {kd: xt_dma(f"x{pr}_{nq}", kd, nq) for kd in range(3)}
                for kd in range(NDT):
                    xt_t = tiles.pop(kd)
                    nc.tensor.matmul(
                        qk_ps[:, 0:SC], wq_sb[:, pr, kd, :],
                        xt_t[:], start=(kd == 0), stop=(kd == NDT - 1),
                    )
                    nc.tensor.matmul(
                        qk_ps[:, SC:2 * SC], wk_sb[:, pr, kd, :],
                        xt_t[:], start=(kd == 0), stop=(kd == NDT - 1),
                    )
                    if kd + 3 < NDT:
                        tiles[kd + 3] = xt_dma(f"x{pr}_{nq}", kd + 3, nq)
                    if kd < NDT - 1:
                        yield
                drain_qk(pr, nq, qk_ps)
                yield

            def out_chunk_gen(st, ndp, tag="aux", tag_bufs=1):
                """Output projection rows st*128.. for d-cols [ndp*1024, +1024)."""
                o_ps = psum.tile([P, 2 * SC], DT.float32, tag=tag, bufs=tag_bufs,
                                 name=f"o{st}_{ndp}")
                for m in range(NMT):
                    nc.tensor.matmul(
                        o_ps[:, 0:SC], ctxt[:, m, st * P:(st + 1) * P],
                        wo_sb[:, m, (2 * ndp) * SC:(2 * ndp + 1) * SC],
                        start=(m == 0), stop=(m == NMT - 1),
                    )
                    nc.tensor.matmul(
                        o_ps[:, SC:2 * SC], ctxt[:, m, st * P:(st + 1) * P],
                        wo_sb[:, m, (2 * ndp + 1) * SC:(2 * ndp + 2) * SC],
                        start=(m == 0), stop=(m == NMT - 1),
                    )
                    if m < NMT - 1:
                        yield
                o_t = outp.tile([P, 2 * SC], DT.float16, tag="out",
                                name=f"ot{st}_{ndp}")
                nc.vector.tensor_copy(o_t[:], o_ps[:])
                nc.sync.dma_start(
                    out[st * P:(st + 1) * P,
                        (2 * ndp) * SC:(2 * ndp + 2) * SC], o_t[:]
                )
                yield

            feeders = collections.deque()
            # next-pair QK chunks, ordered by when their outputs are consumed
            for pr in (1, 2, 3):
                for nq in (0, 3, 2, 1):
                    feeders.append(qk_chunk_gen(pr, nq))

            def pump():
                while feeders:
                    try:
                        next(feeders[0])
                        return True
                    except StopIteration:
                        feeders.popleft()
                return False

            # ---------- attention slots ----------
            def emit_scores(pr, qc, kt, t):
                s01 = psum.tile([P, 2, SC], DT.float32, tag="sc", bufs=2,
                                name=f"s{t}")
                jg0 = qc * SC
                nc.tensor.matmul(
                    s01[:, 0, :], kt_sb[0:64, pr, kt * P:(kt + 1) * P],
                    qt_sb[0:64, pr, jg0:jg0 + SC],
                    start=True, stop=True, tile_position=(0, 0),
                )
                nc.tensor.matmul(
                    s01[:, 1, :], kt_sb[64:128, pr, kt * P:(kt + 1) * P],
                    qt_sb[64:128, pr, jg0:jg0 + SC],
                    start=True, stop=True, tile_position=(64, 0),
                )
                return s01

            def norm_qc(pr, qc):
                """Deferred softmax division for (pair, qc); overlaps later slots."""
                den_sb = stage.tile([2, SC], DT.float32, tag="den2",
                                    name=f"dq{pr}_{qc}")
                rows = [2 * pr * NQC + qc, (2 * pr + 1) * NQC + qc]
                for r, row in enumerate(rows):
                    nc.sync.dma_start(den_sb[r:r + 1, :], den_dram[row, :])
                rcp2 = stage.tile([2, SC], DT.float32, tag="rcp2",
                                  name=f"rq{pr}_{qc}")
                nc.vector.reciprocal_approx_fast(rcp2[:], den_sb[:])
                for r, row in enumerate(rows):
                    nc.sync.dma_start(rcp_dram[row, :], rcp2[r:r + 1, :])
                q0t = S - (qc + 1) * SC
                for r in range(2):
                    hh = 2 * pr + r
                    off = r * 64
                    rb = stage.tile([P, SC], DT.float32, tag="rb",
                                    name=f"rb{hh}_{qc}")
                    bcast = bass.AP(
                        tensor=rcp_dram.tensor,
                        offset=rcp_dram.offset + (hh * NQC + qc) * SC,
                        ap=[[0, D_KV], [1, SC]],
                    )
                    nc.sync.dma_start(rb[off:off + D_KV, :], bcast)
                    cslc = ctxt[off:off + 64, pr, q0t:q0t + SC]
                    nc.gpsimd.tensor_tensor(
                        cslc, cslc, rb[off:off + D_KV, :], OP.mult
                    )

            NTOK = 16 * NKT  # 16 slots x 16 k-tiles
            slots = [(pr, qc) for pr in range(NMT) for qc in range(NQC)]

            def decode(t):
                si, kt = divmod(t, NKT)
                return slots[si][0], slots[si][1], kt

            u_next = None
            pend = {}
            pend[0] = emit_scores(*decode(0), 0)
            pend[1] = emit_scores(*decode(1), 1)

            cx01 = None
            for t in range(NTOK):
                pr, qc, kt = decode(t)
                jg0 = qc * SC
                if kt == 0:
                    if qc == 0 and pr + 1 < NMT:
                        u_next = load_u(pr + 1)
                    cx01 = psum.tile([P, 2 * SC], DT.float32, tag="cx", bufs=1,
                                     name=f"cx{pr}_{qc}")
                s01 = pend.pop(t)
                # ACT: exp(scores/8 + mask_k), psum -> sbuf bf16, both heads
                px = pexpp.tile([P, 2, SC], DT.bfloat16, tag="pexp",
                                name=f"px{t}")
                nc.scalar.activation(
                    out=px[:], in_=s01[:], func=AF.Exp,
                    bias=mask_sb[:, kt:kt + 1], scale=1.0 / math.sqrt(D_KV),
                )
                # DVE: multiply by exp(bias) Toeplitz, both heads in one op
                j0 = kt * P + jg0
                nc.vector.tensor_tensor(
                    px[:], px[:], u_cur[:, :, j0:j0 + SC], OP.mult
                )
                # PE: scores two iterations ahead
                if t + 2 < NTOK:
                    pend[t + 2] = emit_scores(*decode(t + 2), t + 2)
                # feeder matmuls keep the PE saturated past ACT's rate (and
                # give the px chain latency cover before PV needs it)
                pump()
                if kt < 2:
                    pump()
                # PE: PV for both heads (even: [V|ones], odd: [ones|V])
                nc.tensor.matmul(
                    cx01[:, 0:SC], vaug[:, kt, 2 * pr, :], px[:, 0, :],
                    start=(kt == 0), stop=(kt == NKT - 1),
                )
                nc.tensor.matmul(
                    cx01[:, SC:2 * SC], vaug[:, kt, 2 * pr + 1, :], px[:, 1, :],
                    start=(kt == 0), stop=(kt == NKT - 1),
                )
                if kt == NKT - 1:
                    # drain slot: ctx rows + denominator rows (all DVE,
                    # partition-aligned), then deferred normalize
                    cx0, cx1 = cx01[:, 0:SC], cx01[:, SC:2 * SC]
                    # h0 ctx copy on ACT, h1 on DVE: both finish ~0.7us after
                    # PV(15) so the cx psum slot frees before the next slot's
                    # PV(0) arrives
                    dst0 = ctxt[0:64, pr, :]
                    nc.scalar.copy(_rev_ap(dst0, jg0), cx0[0:D_KV, :])
                    dst1 = ctxt[64:128, pr, :]
                    nc.vector.tensor_copy(_rev_ap(dst1, jg0), cx1[64:128, :])
                    dn = stage.tile([P, SC], DT.float32, tag="dn",
                                    name=f"dn{pr}_{qc}")
                    for r, (src_row, dn_row) in enumerate(((64, 64), (0, 0))):
                        csrc = (cx0, cx1)[r][src_row:src_row + 1, :]
                        dslc = dn[dn_row:dn_row + 1, :]
                        drev = bass.AP(
                            tensor=dslc.tensor,
                            offset=dslc.offset + (SC - 1),
                            ap=[list(dslc.ap[0]), [-1, SC]],
                        )
                        nc.vector.tensor_copy(drev, csrc)
                        nc.sync.dma_start(
                            den_dram[(2 * pr + r) * NQC + qc, :],
                            dn[dn_row:dn_row + 1, :],
                        )
                    norm_qc(pr, qc)
                    if pr == NMT - 1:
                        # out-proj rows for this (now fully normalized) q-range.
                        # The last group runs in phase C where the attention
                        # psum tags are free: rotate tags so drains overlap.
                        last = qc == NQC - 1
                        tags = (("sc", 2), ("cx", 1), ("aux", 1))
                        st0 = (S - (qc + 1) * SC) // P
                        for i, (st, ndp) in enumerate(
                            (st, ndp)
                            for st in range(st0, st0 + SC // P)
                            for ndp in range(2)
                        ):
                            tg, tb = tags[i % 3] if last else ("aux", 1)
                            feeders.append(out_chunk_gen(st, ndp, tg, tb))
                    if qc == NQC - 1:
                        u_cur = u_next

            # ---------- phase C: remaining output projection ----------
            while pump():
                pass

    nc.finalize()
    return nc


_NC_CACHE = None


def _get_nc():
    global _NC_CACHE
    if _NC_CACHE is None:
        _NC_CACHE = _build()
    return _NC_CACHE


def _in_maps(hidden_states, attention_mask, Wq, Wk, Wv, Wo, rel_emb):
    import ml_dtypes
    bf16 = ml_dtypes.bfloat16
    # host-side structural gather: exp(rel bias) along score diagonals
    ediag_full = np.exp(rel_emb[_BUCKETS, :].astype(np.float64)).astype(bf16)
    maps = []
    for c in range(NCORES):
        b, g = c // 4, c % 4
        hlo, hhi = g * HL, (g + 1) * HL
        def _wlayout_qk(w):  # [D, hd] -> [p, pr, kt, c] pair-major
            return np.ascontiguousarray(
                w.reshape(NDT, P, NMT, P).transpose(1, 2, 0, 3)
            ).astype(bf16)

        def _wlayout(w):  # [D, hd] -> [(p kt) h] SBUF-matched layout
            return np.ascontiguousarray(
                w.reshape(NDT, P, HL * D_KV).transpose(1, 0, 2).reshape(D, HL * D_KV)
            ).astype(bf16)
        wo_c = Wo[hlo * D_KV:hhi * D_KV, :]
        maps.append({
            "xt": np.ascontiguousarray(hidden_states[b].T).astype(bf16),
            "wq": _wlayout_qk(Wq[:, hlo * D_KV:hhi * D_KV]),
            "wk": _wlayout_qk(Wk[:, hlo * D_KV:hhi * D_KV]),
            "wv": _wlayout(Wv[:, hlo * D_KV:hhi * D_KV]),
            "wo": np.ascontiguousarray(
                wo_c.reshape(NMT, P, D).transpose(1, 0, 2).reshape(HL * D_KV, D)
            ).astype(bf16),
            "mask": np.ascontiguousarray(
                attention_mask[b, 0, 0, :].reshape(NKT, P).T.reshape(S)
            ).astype(np.float32),
            "ediag": np.ascontiguousarray(ediag_full[:, hlo:hhi].T),
        })
    return maps


def kernel(hidden_states, attention_mask, Wq, Wk, Wv, Wo, rel_emb, _trace=False,
           _trace_kwargs=None):
    hidden_states = np.asarray(hidden_states, dtype=np.float32)
    attention_mask = np.asarray(attention_mask, dtype=np.float32)
    Wq = np.asarray(Wq, dtype=np.float32)
    Wk = np.asarray(Wk, dtype=np.float32)
    Wv = np.asarray(Wv, dtype=np.float32)
    Wo = np.asarray(Wo, dtype=np.float32)
    rel_emb = np.asarray(rel_emb, dtype=np.float32)

    nc = _get_nc()
    maps = _in_maps(hidden_states, attention_mask, Wq, Wk, Wv, Wo, rel_emb)
    kw = dict(_trace_kwargs or {})
    res = run_bass_kernel_spmd(nc, maps, core_ids=list(range(NCORES)),
                               trace=_trace, **kw)
    kernel.last_results = res
    outp = np.empty((B, S, D), dtype=np.float32)
    for b in range(B):
        acc = np.asarray(res.results[4 * b]["out"], dtype=np.float32).copy()
        for g in range(1, 4):
            acc += np.asarray(res.results[4 * b + g]["out"], dtype=np.float32)
        outp[b] = acc
    return outp


# revision 23
# speedup vs baseline: 1.0165x; 1.0162x over previous
"""T5-style encoder self-attention (dense_transformer) on 8 Trainium2 NeuronCores.

Problem (full shapes): hidden [2,2048,2048], Wq/Wk/Wv/Wo [2048,2048],
rel_emb [32,32] (bidirectional T5 relative-position bias), mask [2,1,1,2048].

Sharding: data-parallel over batch (2) x tensor-parallel over heads (4 groups
of 8 heads) = 8 cores, Megatron-style. Each core computes a partial output
[2048,2048] (fp16) for its batch; the host sums 4 partials per batch in fp32.

Per-core kernel design (bf16 operands, fp32 PSUM accumulation), organized as
ONE continuous PE instruction stream so the tensor engine never idles (TRN2
p-state: any PE gap drops the clock 2.4->1.2 GHz and costs ~3us of half-rate
execution afterwards):
  - phase A: fused projection chunks. Each 512-column chunk of x^T is
    streamed ONCE and feeds pair-0 Q^T/K^T (lhsT=W slices) AND V for all 8
    heads (lhsT=x^T 128-col slices, rhs=Wv) -- 6 matmuls per xt tile.
  - attention slots (pair, q-chunk): per k-tile iteration the PE executes
    [scores(kt+2) | 2 feeder matmuls | PV(kt)] while ACT runs exp(kt) and
    DVE runs the Toeplitz bias multiply (both heads in ONE 3D-AP tensor op).
    Feeder matmuls are the NEXT pair's Q/K projection chunks (and, during
    pair-3 slots, output-projection chunks), pumped from generators at 1-2
    steps/iter.  PSUM: scores 2-deep (4 banks) + ctx accumulator (2) +
    feeder accumulator (2) = all 8 banks.
  - Q^T is stored with s REVERSED so the relative-position bias is a
    positive-shear Toeplitz; exp(bias) diagonals are HOST-precomputed from
    rel_emb (structural bucket pattern) and DMA'd with sheared APs.
  - softmax without max-subtraction (scores are O(1)); V_aug carries a ones
    block so PSUM rows replicate the denominator for free.  Even heads use
    [V|ones], odd heads [ones|V] so ctx rows land on partitions 0:64 / 64:128
    and every psum->sbuf copy is partition-aligned (runs on DVE).
  - normalization is per-(pair,qc), deferred via a DRAM bounce (reciprocal +
    stride-0 broadcast DMA), fully overlapped with later slots.
  - output projection is chunked (2 x 512-wide psum halves, contraction over
    the 4 head-pairs) and interleaved into pair-3 slots as soon as the
    corresponding q-range is normalized; the fp32->fp16 drain copies run on
    DVE and partial outputs are summed on the host.
"""

import collections
import math
import sys

for _p in ("/opt/trn_rl_repo",):
    if _p not in sys.path:
        sys.path.insert(0, _p)

import numpy as np

import concourse.bass as bass
import concourse.mybir as mybir
import concourse.tile as tile
from concourse import bacc
from concourse.bass_utils import run_bass_kernel_spmd

DT = mybir.dt
AF = mybir.ActivationFunctionType
OP = mybir.AluOpType

# ---- problem constants (hardcoded per contract) ----
B, S, D = 2, 2048, 2048
N_HEADS, D_KV = 32, 64
NUM_BUCKETS, MAX_DISTANCE = 32, 128
NCORES = 8
HL = 8            # heads per core
P = 128
SC = 512          # free-dim chunk
NKT = S // P      # 16 k-tiles
NQC = S // SC     # 4 q-chunks
NDT = D // P      # 16 D-tiles
NMT = (HL * D_KV) // P   # 4 hd m-tiles per core
W_U = 3968        # toeplitz tile width: max(kt*P + qc*SC) + SC
NDIAG = 4096      # ediag row stride (4095 used)


def _rel_bucket_host(d):
    """Exact numpy replica of reference._relative_position_bucket (fp32 math,
    int32 truncation) for bidirectional buckets. d = k - q (int array)."""
    num_buckets = NUM_BUCKETS // 2          # 16
    max_exact = num_buckets // 2            # 8
    rel = np.asarray(d, dtype=np.int64)
    buckets = (rel > 0).astype(np.int32) * num_buckets
    arel = np.abs(rel)
    is_small = arel < max_exact
    rp_safe = np.maximum(arel, 1).astype(np.float32)
    log_ratio = np.log(rp_safe / np.float32(max_exact)).astype(np.float32)
    scale = np.float32(math.log(MAX_DISTANCE / max_exact))
    rp_large = max_exact + (log_ratio / scale * np.float32(num_buckets - max_exact)).astype(np.int32)
    rp_large = np.minimum(rp_large, num_buckets - 1)
    buckets = buckets + np.where(is_small, arel.astype(np.int32), rp_large)
    return buckets.astype(np.int32)


_BUCKETS = _rel_bucket_host(np.arange(NDIAG) - (S - 1))  # diag index -> bucket


def _rev_ap(base, jg0):
    """Reversed-q write AP into a [*, S]-shaped row range (un-reverse)."""
    return bass.AP(
        tensor=base.tensor,
        offset=base.offset + (S - 1 - jg0),
        ap=[list(base.ap[0]), [-1, SC]],
    )


def _build():
    nc = bacc.Bacc(None, name="attn_tp2")

    xt = nc.declare_dram_parameter("xt", [D, S], DT.bfloat16, isOutput=False)
    wq = nc.declare_dram_parameter("wq", [P, NMT, NDT, P], DT.bfloat16, isOutput=False)
    wk = nc.declare_dram_parameter("wk", [P, NMT, NDT, P], DT.bfloat16, isOutput=False)
    wv = nc.declare_dram_parameter("wv", [D, HL * D_KV], DT.bfloat16, isOutput=False)
    wo = nc.declare_dram_parameter("wo", [HL * D_KV, D], DT.bfloat16, isOutput=False)
    mask = nc.declare_dram_parameter("mask", [S], DT.float32, isOutput=False)
    ediag = nc.declare_dram_parameter("ediag", [HL, NDIAG], DT.bfloat16, isOutput=False)
    out = nc.declare_dram_parameter("out", [S, D], DT.float16, isOutput=True)

    with tile.TileContext(nc) as tc:
        with (
            tc.tile_pool(name="res", bufs=1) as res,
            tc.tile_pool(name="xtp", bufs=6) as xtp,
            tc.tile_pool(name="stage", bufs=2) as stage,
            tc.tile_pool(name="upool", bufs=1) as upool,
            tc.tile_pool(name="pexp", bufs=3) as pexpp,
            tc.tile_pool(name="outp", bufs=2) as outp,
            tc.tile_pool(name="psum", bufs=1, space="PSUM") as psum,
            tc.tile_pool(name="dram", bufs=1, space="DRAM") as dramp,
        ):
            # ---------- resident constants / weights ----------
            # hosts pre-transposes weights/mask to the SBUF layout so every
            # DMA below is a contiguous per-partition burst
            mask_sb = res.tile([P, NKT], DT.float32, tag="mask")
            nc.sync.dma_start(mask_sb[:], mask.ap().rearrange("(p kt) -> p kt", p=P))

            # wq/wk are pair-major [p, pr, kd, c] so each pair's block is one
            # contiguous-burst DMA; pair 0 lands first
            wq_sb = res.tile([P, NMT, NDT, P], DT.bfloat16, tag="wq")
            wk_sb = res.tile([P, NMT, NDT, P], DT.bfloat16, tag="wk")
            wv_sb = res.tile([P, NDT, HL * D_KV], DT.bfloat16, tag="wv")
            nc.sync.dma_start(wq_sb[:, 0], wq[:, 0])
            nc.sync.dma_start(wk_sb[:, 0], wk[:, 0])
            # wv split per-kd so V matmuls start as soon as their block lands
            wv_r = wv.ap().rearrange("(p kt) h -> p kt h", p=P)
            for kd in range(NDT):
                nc.sync.dma_start(wv_sb[:, kd, :], wv_r[:, kd, :])

            # preload the ACT exp table early (off the critical path)
            warm = stage.tile([1, 1], DT.float32, tag="warm")
            nc.scalar.activation(out=warm[:], in_=mask_sb[0:1, 0:1], func=AF.Exp)

            wo_sb = res.tile([P, NMT, D], DT.bfloat16, tag="wo")

            # persistent activations
            qt_sb = res.tile([P, NMT, S], DT.bfloat16, tag="qt")   # q REVERSED
            kt_sb = res.tile([P, NMT, S], DT.bfloat16, tag="kt")
            # V_aug: even heads [V | ones], odd heads [ones | V] so ctx rows
            # land on the matching ctxt partitions.
            vaug = res.tile([P, NKT, HL, 2 * D_KV], DT.bfloat16, tag="vaug")
            ctxt = res.tile([P, NMT, S], DT.bfloat16, tag="ctxt")
            nc.vector.memset(vaug[:], 1.0)

            den_dram = dramp.tile([HL * NQC, SC], DT.float32)
            rcp_dram = dramp.tile([HL * NQC, SC], DT.float32)

            # ---------- helpers ----------
            def load_u(pr):
                """Merged Toeplitz exp-bias tile for pair pr: [P, 2, W_U].
                Odd pairs alias the wv slot (dead after phase A) to save SBUF."""
                if pr % 2 == 0:
                    u = upool.tile([P, 2, W_U], DT.bfloat16, tag="u", name=f"u{pr}")
                else:
                    u = res.tile([P, 2, W_U], DT.bfloat16, tag="wv", name=f"u{pr}")
                for i in (0, 1):
                    shear = bass.AP(
                        tensor=ediag.ap().tensor,
                        offset=ediag.ap().offset + (2 * pr + i) * NDIAG,
                        ap=[[1, P], [1, W_U]],
                    )
                    nc.sync.dma_start(u[:, i, :], shear)
                return u

            def xt_dma(tag_name, kd, nq):
                t = xtp.tile([P, SC], DT.bfloat16, tag="xt", name=f"{tag_name}_{kd}")
                nc.sync.dma_start(t[:], xt[kd * P:(kd + 1) * P, nq * SC:(nq + 1) * SC])
                return t

            def drain_qk(pr, nq, qk_ps):
                """Write reversed Q^T and K^T chunks from psum (DVE)."""
                dst = qt_sb[:, pr, :]
                nc.vector.tensor_copy(_rev_ap(dst, nq * SC), qk_ps[:, 0:SC])
                nc.vector.tensor_copy(
                    kt_sb[:, pr, nq * SC:(nq + 1) * SC], qk_ps[:, SC:2 * SC]
                )

            def drain_v(nq, st_pair, v_ps):
                """Scatter V into vaug with per-parity column offsets (DVE)."""
                for j in (0, 1):
                    st = 2 * st_pair + j
                    kt_glob = nq * 4 + st
                    src = v_ps[:, j * SC:(j + 1) * SC].rearrange(
                        "p (h d) -> p h d", d=D_KV
                    )
                    # even heads -> cols 0:64, odd heads -> cols 64:128
                    nc.vector.tensor_copy(
                        vaug[:, kt_glob, 0::2, 0:D_KV], src[:, 0::2, :]
                    )
                    nc.vector.tensor_copy(
                        vaug[:, kt_glob, 1::2, D_KV:2 * D_KV], src[:, 1::2, :]
                    )

            # ---------- phase A: fused pair-0 QK + all-head V ----------
            def phase_a_chunk(nq, next_nq=None):
                qk_ps = psum.tile([P, 2 * SC], DT.float32, tag="sc", bufs=2,
                                  name=f"Aqk{nq}")
                v01 = psum.tile([P, 2 * SC], DT.float32, tag="cx", bufs=1,
                                name=f"Av01_{nq}")
                v23 = psum.tile([P, 2 * SC], DT.float32, tag="aux", bufs=1,
                                name=f"Av23_{nq}")
                v_ps = [v01[:, 0:SC], v01[:, SC:2 * SC],
                        v23[:, 0:SC], v23[:, SC:2 * SC]]
                tiles = dict(a_prefetch) if a_prefetch else {
                    kd: xt_dma(f"Ax{nq}", kd, nq) for kd in range(3)}
                a_prefetch.clear()
                for kd in range(NDT):
                    xt_t = tiles.pop(kd)
                    nc.tensor.matmul(
                        qk_ps[:, 0:SC], wq_sb[:, 0, kd, :], xt_t[:],
                        start=(kd == 0), stop=(kd == NDT - 1),
                    )
                    nc.tensor.matmul(
                        qk_ps[:, SC:2 * SC], wk_sb[:, 0, kd, :], xt_t[:],
                        start=(kd == 0), stop=(kd == NDT - 1),
                    )
                    for st in range(4):
                        nc.tensor.matmul(
                            v_ps[st], xt_t[:, st * P:(st + 1) * P],
                            wv_sb[:, kd, :],
                            start=(kd == 0), stop=(kd == NDT - 1),
                        )
                    if kd + 3 < NDT:
                        tiles[kd + 3] = xt_dma(f"Ax{nq}", kd + 3, nq)
                    elif next_nq is not None:
                        k2 = kd + 3 - NDT
                        a_prefetch[k2] = xt_dma(f"Ax{next_nq}", k2, next_nq)
                drain_qk(0, nq, qk_ps)
                drain_v(nq, 0, v01)
                drain_v(nq, 1, v23)

            # bulk DMAs staged between chunks, earliest-needed first, so
            # they never sit ahead of the next chunk's xt stream
            a_prefetch = {}
            phase_a_chunk(0, next_nq=1)
            u_cur = load_u(0)
            phase_a_chunk(1, next_nq=2)
            nc.sync.dma_start(wq_sb[:, 1], wq[:, 1])
            nc.sync.dma_start(wk_sb[:, 1], wk[:, 1])
            phase_a_chunk(2, next_nq=3)
            nc.sync.dma_start(wq_sb[:, 2:4], wq[:, 2:4])
            nc.sync.dma_start(wk_sb[:, 2:4], wk[:, 2:4])
            phase_a_chunk(3)
            nc.sync.dma_start(wo_sb[:], wo.ap().rearrange("(p mt) d -> p mt d", p=P))

            # ---------- feeder generators ----------
            def qk_chunk_gen(pr, nq):
                qk_ps = psum.tile([P, 2 * SC], DT.float32, tag="aux", bufs=1,
                                  name=f"qk{pr}_{nq}")
                tiles = {kd: xt_dma(f"x{pr}_{nq}", kd, nq) for kd in range(3)}
                for kd in range(NDT):
                    xt_t = tiles.pop(kd)
                    nc.tensor.matmul(
                        qk_ps[:, 0:SC], wq_sb[:, pr, kd, :],
                        xt_t[:], start=(kd == 0), stop=(kd == NDT - 1),
                    )
                    nc.tensor.matmul(
                        qk_ps[:, SC:2 * SC], wk_sb[:, pr, kd, :],
                        xt_t[:], start=(kd == 0), stop=(kd == NDT - 1),
                    )
                    if kd + 3 < NDT:
                        tiles[kd + 3] = xt_dma(f"x{pr}_{nq}", kd + 3, nq)
                    if kd < NDT - 1:
                        yield
                drain_qk(pr, nq, qk_ps)
                yield

            def out_chunk_gen(st, ndp, tag="aux", tag_bufs=1):
                """Output projection rows st*128.. for d-cols [ndp*1024, +1024)."""
                o_ps = psum.tile([P, 2 * SC], DT.float32, tag=tag, bufs=tag_bufs,
                                 name=f"o{st}_{ndp}")
                for m in range(NMT):
                    nc.tensor.matmul(
                        o_ps[:, 0:SC], ctxt[:, m, st * P:(st + 1) * P],
                        wo_sb[:, m, (2 * ndp) * SC:(2 * ndp + 1) * SC],
                        start=(m == 0), stop=(m == NMT - 1),
                    )
                    nc.tensor.matmul(
                        o_ps[:, SC:2 * SC], ctxt[:, m, st * P:(st + 1) * P],
                        wo_sb[:, m, (2 * ndp + 1) * SC:(2 * ndp + 2) * SC],
                        start=(m == 0), stop=(m == NMT - 1),
                    )
                    if m < NMT - 1:
                        yield
                o_t = outp.tile([P, 2 * SC], DT.float16, tag="out",
                                name=f"ot{st}_{ndp}")
                nc.vector.tensor_copy(o_t[:], o_ps[:])
                nc.sync.dma_start(
                    out[st * P:(st + 1) * P,
                        (2 * ndp) * SC:(2 * ndp + 2) * SC], o_t[:]
                )
                yield

            feeders = collections.deque()
            # next-pair QK chunks, ordered by when their outputs are consumed
            for pr in (1, 2, 3):
                for nq in (0, 3, 2, 1):
                    feeders.append(qk_chunk_gen(pr, nq))

            def pump():
                while feeders:
                    try:
                        next(feeders[0])
                        return True
                    except StopIteration:
                        feeders.popleft()
                return False

            # ---------- attention slots ----------
            def emit_scores(pr, qc, kt, t):
                s01 = psum.tile([P, 2, SC], DT.float32, tag="sc", bufs=2,
                                name=f"s{t}")
                jg0 = qc * SC
                nc.tensor.matmul(
                    s01[:, 0, :], kt_sb[0:64, pr, kt * P:(kt + 1) * P],
                    qt_sb[0:64, pr, jg0:jg0 + SC],
                    start=True, stop=True, tile_position=(0, 0),
                )
                nc.tensor.matmul(
                    s01[:, 1, :], kt_sb[64:128, pr, kt * P:(kt + 1) * P],
                    qt_sb[64:128, pr, jg0:jg0 + SC],
                    start=True, stop=True, tile_position=(64, 0),
                )
                return s01

            def norm_qc(pr, qc):
                """Deferred softmax division for (pair, qc); overlaps later slots."""
                den_sb = stage.tile([2, SC], DT.float32, tag="den2",
                                    name=f"dq{pr}_{qc}")
                rows = [2 * pr * NQC + qc, (2 * pr + 1) * NQC + qc]
                for r, row in enumerate(rows):
                    nc.sync.dma_start(den_sb[r:r + 1, :], den_dram[row, :])
                rcp2 = stage.tile([2, SC], DT.float32, tag="rcp2",
                                  name=f"rq{pr}_{qc}")
                nc.vector.reciprocal_approx_fast(rcp2[:], den_sb[:])
                for r, row in enumerate(rows):
                    nc.sync.dma_start(rcp_dram[row, :], rcp2[r:r + 1, :])
                q0t = S - (qc + 1) * SC
                for r in range(2):
                    hh = 2 * pr + r
                    off = r * 64
                    rb = stage.tile([P, SC], DT.float32, tag="rb",
                                    name=f"rb{hh}_{qc}")
                    bcast = bass.AP(
                        tensor=rcp_dram.tensor,
                        offset=rcp_dram.offset + (hh * NQC + qc) * SC,
                        ap=[[0, D_KV], [1, SC]],
                    )
                    nc.sync.dma_start(rb[off:off + D_KV, :], bcast)
                    cslc = ctxt[off:off + 64, pr, q0t:q0t + SC]
                    nc.gpsimd.tensor_tensor(
                        cslc, cslc, rb[off:off + D_KV, :], OP.mult
                    )

            NTOK = 16 * NKT  # 16 slots x 16 k-tiles
            slots = [(pr, qc) for pr in range(NMT) for qc in range(NQC)]

            def decode(t):
                si, kt = divmod(t, NKT)
                return slots[si][0], slots[si][1], kt

            u_next = None
            pend = {}
            pend[0] = emit_scores(*decode(0), 0)
            pend[1] = emit_scores(*decode(1), 1)

            cx01 = None
            for t in range(NTOK):
                pr, qc, kt = decode(t)
                jg0 = qc * SC
                if kt == 0:
                    if qc == 0 and pr + 1 < NMT:
                        u_next = load_u(pr + 1)
                    cx01 = psum.tile([P, 2 * SC], DT.float32, tag="cx", bufs=1,
                                     name=f"cx{pr}_{qc}")
                s01 = pend.pop(t)
                # ACT: exp(scores/8 + mask_k), psum -> sbuf bf16, both heads
                px = pexpp.tile([P, 2, SC], DT.bfloat16, tag="pexp",
                                name=f"px{t}")
                nc.scalar.activation(
                    out=px[:], in_=s01[:], func=AF.Exp,
                    bias=mask_sb[:, kt:kt + 1], scale=1.0 / math.sqrt(D_KV),
                )
                # DVE: multiply by exp(bias) Toeplitz, both heads in one op
                j0 = kt * P + jg0
                nc.vector.tensor_tensor(
                    px[:], px[:], u_cur[:, :, j0:j0 + SC], OP.mult
                )
                # PE: scores two iterations ahead
                if t + 2 < NTOK:
                    pend[t + 2] = emit_scores(*decode(t + 2), t + 2)
                # feeder matmuls keep the PE saturated past ACT's rate (and
                # give the px chain latency cover before PV needs it)
                pump()
                if kt < 2:
                    pump()
                # PE: PV for both heads (even: [V|ones], odd: [ones|V])
                nc.tensor.matmul(
                    cx01[:, 0:SC], vaug[:, kt, 2 * pr, :], px[:, 0, :],
                    start=(kt == 0), stop=(kt == NKT - 1),
                )
                nc.tensor.matmul(
                    cx01[:, SC:2 * SC], vaug[:, kt, 2 * pr + 1, :], px[:, 1, :],
                    start=(kt == 0), stop=(kt == NKT - 1),
                )
                if kt == NKT - 1:
                    # drain slot: ctx rows + denominator rows (all DVE,
                    # partition-aligned), then deferred normalize
                    cx0, cx1 = cx01[:, 0:SC], cx01[:, SC:2 * SC]
                    # h0 ctx copy on ACT, h1 on DVE: both finish ~0.7us after
                    # PV(15) so the cx psum slot frees before the next slot's
                    # PV(0) arrives
                    dst0 = ctxt[0:64, pr, :]
                    nc.scalar.copy(_rev_ap(dst0, jg0), cx0[0:D_KV, :])
                    dst1 = ctxt[64:128, pr, :]
                    nc.vector.tensor_copy(_rev_ap(dst1, jg0), cx1[64:128, :])
                    dn = stage.tile([P, SC], DT.float32, tag="dn",
                                    name=f"dn{pr}_{qc}")
                    for r, (src_row, dn_row) in enumerate(((64, 64), (0, 0))):
                        csrc = (cx0, cx1)[r][src_row:src_row + 1, :]
                        dslc = dn[dn_row:dn_row + 1, :]
                        drev = bass.AP(
                            tensor=dslc.tensor,
                            offset=dslc.offset + (SC - 1),
                            ap=[list(dslc.ap[0]), [-1, SC]],
                        )
                        nc.vector.tensor_copy(drev, csrc)
                        nc.sync.dma_start(
                            den_dram[(2 * pr + r) * NQC + qc, :],
                            dn[dn_row:dn_row + 1, :],
                        )
                    norm_qc(pr, qc)
                    if pr == NMT - 1:
                        # out-proj rows for this (now fully normalized) q-range.
                        # The last group runs in phase C where the attention
                        # psum tags are free: rotate tags so drains overlap.
                        last = qc == NQC - 1
                        tags = (("sc", 2), ("cx", 1), ("aux", 1))
                        st0 = (S - (qc + 1) * SC) // P
                        for i, (st, ndp) in enumerate(
                            (st, ndp)
                            for st in range(st0, st0 + SC // P)
                            for ndp in range(2)
                        ):
                            tg, tb = tags[i % 3] if last else ("aux", 1)
                            feeders.append(out_chunk_gen(st, ndp, tg, tb))
                    if qc == NQC - 1:
                        u_cur = u_next

            # ---------- phase C: remaining output projection ----------
            while pump():
                pass

    nc.finalize()
    return nc


_NC_CACHE = None


def _get_nc():
    global _NC_CACHE
    if _NC_CACHE is None:
        _NC_CACHE = _build()
    return _NC_CACHE


def _in_maps(hidden_states, attention_mask, Wq, Wk, Wv, Wo, rel_emb):
    import ml_dtypes
    bf16 = ml_dtypes.bfloat16
    # host-side structural gather: exp(rel bias) along score diagonals
    ediag_full = np.exp(rel_emb[_BUCKETS, :].astype(np.float64)).astype(bf16)
    maps = []
    for c in range(NCORES):
        b, g = c // 4, c % 4
        hlo, hhi = g * HL, (g + 1) * HL
        def _wlayout_qk(w):  # [D, hd] -> [p, pr, kt, c] pair-major
            return np.ascontiguousarray(
                w.reshape(NDT, P, NMT, P).transpose(1, 2, 0, 3)
            ).astype(bf16)

        def _wlayout(w):  # [D, hd] -> [(p kt) h] SBUF-matched layout
            return np.ascontiguousarray(
                w.reshape(NDT, P, HL * D_KV).transpose(1, 0, 2).reshape(D, HL * D_KV)
            ).astype(bf16)
        wo_c = Wo[hlo * D_KV:hhi * D_KV, :]
        maps.append({
            "xt": np.ascontiguousarray(hidden_states[b].T).astype(bf16),
            "wq": _wlayout_qk(Wq[:, hlo * D_KV:hhi * D_KV]),
            "wk": _wlayout_qk(Wk[:, hlo * D_KV:hhi * D_KV]),
            "wv": _wlayout(Wv[:, hlo * D_KV:hhi * D_KV]),
            "wo": np.ascontiguousarray(
                wo_c.reshape(NMT, P, D).transpose(1, 0, 2).reshape(HL * D_KV, D)
            ).astype(bf16),
            "mask": np.ascontiguousarray(
                attention_mask[b, 0, 0, :].reshape(NKT, P).T.reshape(S)
            ).astype(np.float32),
            "ediag": np.ascontiguousarray(ediag_full[:, hlo:hhi].T),
        })
    return maps


def kernel(hidden_states, attention_mask, Wq, Wk, Wv, Wo, rel_emb, _trace=False,
           _trace_kwargs=None):
    hidden_states = np.asarray(hidden_states, dtype=np.float32)
    attention_mask = np.asarray(attention_mask, dtype=np.float32)
    Wq = np.asarray(Wq, dtype=np.float32)
    Wk = np.asarray(Wk, dtype=np.float32)
    Wv = np.asarray(Wv, dtype=np.float32)
    Wo = np.asarray(Wo, dtype=np.float32)
    rel_emb = np.asarray(rel_emb, dtype=np.float32)

    nc = _get_nc()
    maps = _in_maps(hidden_states, attention_mask, Wq, Wk, Wv, Wo, rel_emb)
    kw = dict(_trace_kwargs or {})
    res = run_bass_kernel_spmd(nc, maps, core_ids=list(range(NCORES)),
                               trace=_trace, **kw)
    kernel.last_results = res
    outp = np.empty((B, S, D), dtype=np.float32)
    for b in range(B):
        acc = np.asarray(res.results[4 * b]["out"], dtype=np.float32).copy()
        for g in range(1, 4):
            acc += np.asarray(res.results[4 * b + g]["out"], dtype=np.float32)
        outp[b] = acc
    return outp


# revision 29
# speedup vs baseline: 1.0965x; 1.0787x over previous
"""T5-style encoder self-attention (dense_transformer) on 8 Trainium2 NeuronCores.

Problem (full shapes): hidden [2,2048,2048], Wq/Wk/Wv/Wo [2048,2048],
rel_emb [32,32] (bidirectional T5 relative-position bias), mask [2,1,1,2048].

Sharding: data-parallel over batch (2) x tensor-parallel over heads (4 groups
of 8 heads) = 8 cores, Megatron-style. Each core computes a partial output
[2048,2048] (fp16) for its batch; the host sums 4 partials per batch in fp32.

Per-core kernel design (bf16 operands, fp32 PSUM accumulation), organized as
ONE continuous PE instruction stream so the tensor engine never idles (TRN2
p-state: any PE gap drops the clock 2.4->1.2 GHz and costs ~3us of half-rate
execution afterwards):
  - phase A: fused projection chunks. Each 512-column chunk of x^T is
    streamed ONCE and feeds pair-0 Q^T/K^T (lhsT=W slices) AND V for all 8
    heads (lhsT=x^T 128-col slices, rhs=Wv) -- 6 matmuls per xt tile.
  - attention slots (pair, q-chunk): per k-tile iteration the PE executes
    [scores(kt+2) | 2 feeder matmuls | PV(kt)] while ACT runs exp(kt) and
    DVE runs the Toeplitz bias multiply (both heads in ONE 3D-AP tensor op).
    Feeder matmuls are the NEXT pair's Q/K projection chunks (and, during
    pair-3 slots, output-projection chunks), pumped from generators at 1-2
    steps/iter.  PSUM: scores 2-deep (4 banks) + ctx accumulator (2) +
    feeder accumulator (2) = all 8 banks.
  - Q^T is stored with s REVERSED so the relative-position bias is a
    positive-shear Toeplitz; exp(bias) diagonals are HOST-precomputed from
    rel_emb (structural bucket pattern) and DMA'd with sheared APs.
  - softmax without max-subtraction (scores are O(1)); V_aug carries a ones
    block so PSUM rows replicate the denominator for free.  Even heads use
    [V|ones], odd heads [ones|V] so ctx rows land on partitions 0:64 / 64:128
    and every psum->sbuf copy is partition-aligned (runs on DVE).
  - normalization is per-(pair,qc), deferred via a DRAM bounce (reciprocal +
    stride-0 broadcast DMA), fully overlapped with later slots.
  - output projection is chunked (2 x 512-wide psum halves, contraction over
    the 4 head-pairs) and interleaved into pair-3 slots as soon as the
    corresponding q-range is normalized; the fp32->fp16 drain copies run on
    DVE and partial outputs are summed on the host.
"""

import collections
import math
import sys

for _p in ("/opt/trn_rl_repo",):
    if _p not in sys.path:
        sys.path.insert(0, _p)

import numpy as np

import concourse.bass as bass
import concourse.mybir as mybir
import concourse.tile as tile
from concourse import bacc
from concourse.bass_utils import run_bass_kernel_spmd

DT = mybir.dt
AF = mybir.ActivationFunctionType
OP = mybir.AluOpType

# ---- problem constants (hardcoded per contract) ----
B, S, D = 2, 2048, 2048
N_HEADS, D_KV = 32, 64
NUM_BUCKETS, MAX_DISTANCE = 32, 128
NCORES = 8
HL = 8            # heads per core
P = 128
SC = 512          # free-dim chunk
NKT = S // P      # 16 k-tiles
NQC = S // SC     # 4 q-chunks
NDT = D // P      # 16 D-tiles
NMT = (HL * D_KV) // P   # 4 hd m-tiles per core
W_U = 3968        # toeplitz tile width: max(kt*P + qc*SC) + SC
NDIAG = 4096      # ediag row stride (4095 used)


def _rel_bucket_host(d):
    """Exact numpy replica of reference._relative_position_bucket (fp32 math,
    int32 truncation) for bidirectional buckets. d = k - q (int array)."""
    num_buckets = NUM_BUCKETS // 2          # 16
    max_exact = num_buckets // 2            # 8
    rel = np.asarray(d, dtype=np.int64)
    buckets = (rel > 0).astype(np.int32) * num_buckets
    arel = np.abs(rel)
    is_small = arel < max_exact
    rp_safe = np.maximum(arel, 1).astype(np.float32)
    log_ratio = np.log(rp_safe / np.float32(max_exact)).astype(np.float32)
    scale = np.float32(math.log(MAX_DISTANCE / max_exact))
    rp_large = max_exact + (log_ratio / scale * np.float32(num_buckets - max_exact)).astype(np.int32)
    rp_large = np.minimum(rp_large, num_buckets - 1)
    buckets = buckets + np.where(is_small, arel.astype(np.int32), rp_large)
    return buckets.astype(np.int32)


_BUCKETS = _rel_bucket_host(np.arange(NDIAG) - (S - 1))  # diag index -> bucket


def _rev_ap(base, jg0):
    """Reversed-q write AP into a [*, S]-shaped row range (un-reverse)."""
    return bass.AP(
        tensor=base.tensor,
        offset=base.offset + (S - 1 - jg0),
        ap=[list(base.ap[0]), [-1, SC]],
    )


def _build():
    nc = bacc.Bacc(None, name="attn_tp2")

    xt = nc.declare_dram_parameter("xt", [D, S], DT.bfloat16, isOutput=False)
    wq = nc.declare_dram_parameter("wq", [P, NMT, NDT, P], DT.bfloat16, isOutput=False)
    wk = nc.declare_dram_parameter("wk", [P, NMT, NDT, P], DT.bfloat16, isOutput=False)
    wv = nc.declare_dram_parameter("wv", [D, HL * D_KV], DT.bfloat16, isOutput=False)
    wo = nc.declare_dram_parameter("wo", [HL * D_KV, D], DT.bfloat16, isOutput=False)
    mask = nc.declare_dram_parameter("mask", [S], DT.float32, isOutput=False)
    ediag = nc.declare_dram_parameter("ediag", [HL, NDIAG], DT.bfloat16, isOutput=False)
    out = nc.declare_dram_parameter("out", [S, D], DT.float16, isOutput=True)

    with tile.TileContext(nc) as tc:
        with (
            tc.tile_pool(name="res", bufs=1) as res,
            tc.tile_pool(name="xtp", bufs=6) as xtp,
            tc.tile_pool(name="stage", bufs=2) as stage,
            tc.tile_pool(name="upool", bufs=1) as upool,
            tc.tile_pool(name="pexp", bufs=3) as pexpp,
            tc.tile_pool(name="outp", bufs=2) as outp,
            tc.tile_pool(name="psum", bufs=1, space="PSUM") as psum,
            tc.tile_pool(name="dram", bufs=1, space="DRAM") as dramp,
        ):
            # ---------- resident constants / weights ----------
            # hosts pre-transposes weights/mask to the SBUF layout so every
            # DMA below is a contiguous per-partition burst
            mask_sb = res.tile([P, NKT], DT.float32, tag="mask")
            nc.sync.dma_start(mask_sb[:], mask.ap().rearrange("(p kt) -> p kt", p=P))

            # wq/wk are pair-major [p, pr, kd, c] so each pair's block is one
            # contiguous-burst DMA; pair 0 lands first
            wq_sb = res.tile([P, NMT, NDT, P], DT.bfloat16, tag="wq")
            wk_sb = res.tile([P, NMT, NDT, P], DT.bfloat16, tag="wk")
            wv_sb = res.tile([P, NDT, HL * D_KV], DT.bfloat16, tag="wv")
            nc.sync.dma_start(wq_sb[:, 0], wq[:, 0])
            nc.sync.dma_start(wk_sb[:, 0], wk[:, 0])
            # wv split per-kd so V matmuls start as soon as their block lands
            wv_r = wv.ap().rearrange("(p kt) h -> p kt h", p=P)
            for kd in range(NDT):
                nc.sync.dma_start(wv_sb[:, kd, :], wv_r[:, kd, :])

            # preload the ACT exp table early (off the critical path)
            warm = stage.tile([1, 1], DT.float32, tag="warm")
            nc.scalar.activation(out=warm[:], in_=mask_sb[0:1, 0:1], func=AF.Exp)

            wo_sb = res.tile([P, NMT, D], DT.bfloat16, tag="wo")

            # persistent activations
            qt_sb = res.tile([P, NMT, S], DT.bfloat16, tag="qt")   # q REVERSED
            kt_sb = res.tile([P, NMT, S], DT.bfloat16, tag="kt")
            # V_aug: even heads [V | ones], odd heads [ones | V] so ctx rows
            # land on the matching ctxt partitions.
            vaug = res.tile([P, NKT, HL, 2 * D_KV], DT.bfloat16, tag="vaug")
            ctxt = res.tile([P, NMT, S], DT.bfloat16, tag="ctxt")
            nc.vector.memset(vaug[:], 1.0)

            den_dram = dramp.tile([HL * NQC, SC], DT.float32)
            rcp_dram = dramp.tile([HL * NQC, SC], DT.float32)

            # ---------- helpers ----------
            def load_u(pr):
                """Merged Toeplitz exp-bias tile for pair pr: [P, 2, W_U].
                Odd pairs alias the wv slot (dead after phase A) to save SBUF."""
                if pr % 2 == 0:
                    u = upool.tile([P, 2, W_U], DT.bfloat16, tag="u", name=f"u{pr}")
                else:
                    u = res.tile([P, 2, W_U], DT.bfloat16, tag="wv", name=f"u{pr}")
                for i in (0, 1):
                    shear = bass.AP(
                        tensor=ediag.ap().tensor,
                        offset=ediag.ap().offset + (2 * pr + i) * NDIAG,
                        ap=[[1, P], [1, W_U]],
                    )
                    nc.sync.dma_start(u[:, i, :], shear)
                return u

            def xt_dma(tag_name, kd, nq):
                t = xtp.tile([P, SC], DT.bfloat16, tag="xt", name=f"{tag_name}_{kd}")
                nc.sync.dma_start(t[:], xt[kd * P:(kd + 1) * P, nq * SC:(nq + 1) * SC])
                return t

            def drain_qk(pr, nq, qk_ps):
                """Write reversed Q^T and K^T chunks from psum (DVE)."""
                dst = qt_sb[:, pr, :]
                nc.vector.tensor_copy(_rev_ap(dst, nq * SC), qk_ps[:, 0:SC])
                nc.vector.tensor_copy(
                    kt_sb[:, pr, nq * SC:(nq + 1) * SC], qk_ps[:, SC:2 * SC]
                )

            def drain_v(nq, st_pair, v_ps, eng):
                """Scatter V into vaug with per-parity column offsets."""
                copy = nc.scalar.copy if eng == "act" else nc.vector.tensor_copy
                for j in (0, 1):
                    st = 2 * st_pair + j
                    kt_glob = nq * 4 + st
                    src = v_ps[:, j * SC:(j + 1) * SC].rearrange(
                        "p (h d) -> p h d", d=D_KV
                    )
                    # even heads -> cols 0:64, odd heads -> cols 64:128
                    copy(vaug[:, kt_glob, 0::2, 0:D_KV], src[:, 0::2, :])
                    copy(vaug[:, kt_glob, 1::2, D_KV:2 * D_KV], src[:, 1::2, :])

            # ---------- phase A: fused pair-0 QK + all-head V ----------
            def phase_a_chunk(nq, next_nq=None):
                qk_ps = psum.tile([P, 2 * SC], DT.float32, tag="sc", bufs=2,
                                  name=f"Aqk{nq}")
                v01 = psum.tile([P, 2 * SC], DT.float32, tag="cx", bufs=1,
                                name=f"Av01_{nq}")
                v23 = psum.tile([P, 2 * SC], DT.float32, tag="aux", bufs=1,
                                name=f"Av23_{nq}")
                v_ps = [v01[:, 0:SC], v01[:, SC:2 * SC],
                        v23[:, 0:SC], v23[:, SC:2 * SC]]
                tiles = dict(a_prefetch) if a_prefetch else {
                    kd: xt_dma(f"Ax{nq}", kd, nq) for kd in range(3)}
                a_prefetch.clear()
                for kd in range(NDT):
                    xt_t = tiles.pop(kd)
                    nc.tensor.matmul(
                        qk_ps[:, 0:SC], wq_sb[:, 0, kd, :], xt_t[:],
                        start=(kd == 0), stop=(kd == NDT - 1),
                    )
                    nc.tensor.matmul(
                        qk_ps[:, SC:2 * SC], wk_sb[:, 0, kd, :], xt_t[:],
                        start=(kd == 0), stop=(kd == NDT - 1),
                    )
                    for st in range(4):
                        nc.tensor.matmul(
                            v_ps[st], xt_t[:, st * P:(st + 1) * P],
                            wv_sb[:, kd, :],
                            start=(kd == 0), stop=(kd == NDT - 1),
                        )
                    if kd + 3 < NDT:
                        tiles[kd + 3] = xt_dma(f"Ax{nq}", kd + 3, nq)
                    elif next_nq is not None:
                        k2 = kd + 3 - NDT
                        a_prefetch[k2] = xt_dma(f"Ax{next_nq}", k2, next_nq)
                drain_v(nq, 0, v01, "act")
                drain_v(nq, 1, v23, "dve")
                drain_qk(0, nq, qk_ps)

            # bulk DMAs staged between chunks, earliest-needed first, so
            # they never sit ahead of the next chunk's xt stream
            a_prefetch = {}
            phase_a_chunk(0, next_nq=1)
            u_cur = load_u(0)
            phase_a_chunk(1, next_nq=2)
            nc.sync.dma_start(wq_sb[:, 1], wq[:, 1])
            nc.sync.dma_start(wk_sb[:, 1], wk[:, 1])
            phase_a_chunk(2, next_nq=3)
            nc.sync.dma_start(wq_sb[:, 2:4], wq[:, 2:4])
            nc.sync.dma_start(wk_sb[:, 2:4], wk[:, 2:4])
            phase_a_chunk(3)
            nc.sync.dma_start(wo_sb[:], wo.ap().rearrange("(p mt) d -> p mt d", p=P))

            # ---------- feeder generators ----------
            def qk_chunk_gen(pr, nq):
                qk_ps = psum.tile([P, 2 * SC], DT.float32, tag="aux", bufs=1,
                                  name=f"qk{pr}_{nq}")
                tiles = {kd: xt_dma(f"x{pr}_{nq}", kd, nq) for kd in range(3)}
                for kd in range(NDT):
                    xt_t = tiles.pop(kd)
                    nc.tensor.matmul(
                        qk_ps[:, 0:SC], wq_sb[:, pr, kd, :],
                        xt_t[:], start=(kd == 0), stop=(kd == NDT - 1),
                    )
                    nc.tensor.matmul(
                        qk_ps[:, SC:2 * SC], wk_sb[:, pr, kd, :],
                        xt_t[:], start=(kd == 0), stop=(kd == NDT - 1),
                    )
                    if kd + 3 < NDT:
                        tiles[kd + 3] = xt_dma(f"x{pr}_{nq}", kd + 3, nq)
                    yield
                # drain as two single-op steps so they never pile up ahead of
                # the attention bias-multiply in the DVE queue
                dst = qt_sb[:, pr, :]
                nc.vector.tensor_copy(_rev_ap(dst, nq * SC), qk_ps[:, 0:SC])
                yield
                nc.vector.tensor_copy(
                    kt_sb[:, pr, nq * SC:(nq + 1) * SC], qk_ps[:, SC:2 * SC]
                )

            phase_c = [False]     # set once the attention token loop is done
            out_rot = [0]
            out_drain_alt = [0]
            OUT_TAGS = (("sc", 2), ("sc", 2), ("cx", 1), ("aux", 1))

            def out_chunk_gen(st, ndp):
                """Output projection rows st*128.. for d-cols [ndp*1024, +1024).
                In phase C the attention psum tags are free: rotate across them
                so chunk drains overlap the next chunk's matmuls."""
                if phase_c[0]:
                    tag, tag_bufs = OUT_TAGS[out_rot[0] % 4]
                    out_rot[0] += 1
                else:
                    tag, tag_bufs = "aux", 1
                o_ps = psum.tile([P, 2 * SC], DT.float32, tag=tag, bufs=tag_bufs,
                                 name=f"o{st}_{ndp}")
                for m in range(NMT):
                    nc.tensor.matmul(
                        o_ps[:, 0:SC], ctxt[:, m, st * P:(st + 1) * P],
                        wo_sb[:, m, (2 * ndp) * SC:(2 * ndp + 1) * SC],
                        start=(m == 0), stop=(m == NMT - 1),
                    )
                    nc.tensor.matmul(
                        o_ps[:, SC:2 * SC], ctxt[:, m, st * P:(st + 1) * P],
                        wo_sb[:, m, (2 * ndp + 1) * SC:(2 * ndp + 2) * SC],
                        start=(m == 0), stop=(m == NMT - 1),
                    )
                    yield
                o_t = outp.tile([P, 2 * SC], DT.float16, tag="out",
                                name=f"ot{st}_{ndp}")
                # alternate the fp32->fp16 drain between DVE and ACT so
                # back-to-back chunks in phase C overlap their drains
                if out_drain_alt[0] % 2:
                    nc.scalar.copy(o_t[:], o_ps[:])
                else:
                    nc.vector.tensor_copy(o_t[:], o_ps[:])
                out_drain_alt[0] += 1
                yield
                nc.sync.dma_start(
                    out[st * P:(st + 1) * P,
                        (2 * ndp) * SC:(2 * ndp + 2) * SC], o_t[:]
                )

            feeders = collections.deque()
            # next-pair QK chunks, ordered by when their outputs are consumed
            for pr in (1, 2, 3):
                for nq in (0, 3, 2, 1):
                    feeders.append(qk_chunk_gen(pr, nq))

            def pump():
                while feeders:
                    try:
                        next(feeders[0])
                        return True
                    except StopIteration:
                        feeders.popleft()
                return False

            # ---------- attention slots ----------
            def emit_scores(pr, qc, kt, t):
                s01 = psum.tile([P, 2, SC], DT.float32, tag="sc", bufs=2,
                                name=f"s{t}")
                jg0 = qc * SC
                nc.tensor.matmul(
                    s01[:, 0, :], kt_sb[0:64, pr, kt * P:(kt + 1) * P],
                    qt_sb[0:64, pr, jg0:jg0 + SC],
                    start=True, stop=True, tile_position=(0, 0),
                )
                nc.tensor.matmul(
                    s01[:, 1, :], kt_sb[64:128, pr, kt * P:(kt + 1) * P],
                    qt_sb[64:128, pr, jg0:jg0 + SC],
                    start=True, stop=True, tile_position=(64, 0),
                )
                return s01

            def norm_qc(pr, qc):
                """Deferred softmax division for (pair, qc); overlaps later slots."""
                den_sb = stage.tile([2, SC], DT.float32, tag="den2",
                                    name=f"dq{pr}_{qc}")
                rows = [2 * pr * NQC + qc, (2 * pr + 1) * NQC + qc]
                for r, row in enumerate(rows):
                    nc.sync.dma_start(den_sb[r:r + 1, :], den_dram[row, :])
                rcp2 = stage.tile([2, SC], DT.float32, tag="rcp2",
                                  name=f"rq{pr}_{qc}")
                nc.vector.reciprocal_approx_fast(rcp2[:], den_sb[:])
                for r, row in enumerate(rows):
                    nc.sync.dma_start(rcp_dram[row, :], rcp2[r:r + 1, :])
                q0t = S - (qc + 1) * SC
                for r in range(2):
                    hh = 2 * pr + r
                    off = r * 64
                    rb = stage.tile([P, SC], DT.float32, tag="rb",
                                    name=f"rb{hh}_{qc}")
                    bcast = bass.AP(
                        tensor=rcp_dram.tensor,
                        offset=rcp_dram.offset + (hh * NQC + qc) * SC,
                        ap=[[0, D_KV], [1, SC]],
                    )
                    nc.sync.dma_start(rb[off:off + D_KV, :], bcast)
                    cslc = ctxt[off:off + 64, pr, q0t:q0t + SC]
                    nc.gpsimd.tensor_tensor(
                        cslc, cslc, rb[off:off + D_KV, :], OP.mult
                    )

            NTOK = 16 * NKT  # 16 slots x 16 k-tiles
            slots = [(pr, qc) for pr in range(NMT) for qc in range(NQC)]

            def decode(t):
                si, kt = divmod(t, NKT)
                return slots[si][0], slots[si][1], kt

            u_next = None
            pend = {}
            pend[0] = emit_scores(*decode(0), 0)
            pend[1] = emit_scores(*decode(1), 1)

            cx01 = None
            for t in range(NTOK):
                pr, qc, kt = decode(t)
                jg0 = qc * SC
                if kt == 0:
                    if qc == 0 and pr + 1 < NMT:
                        u_next = load_u(pr + 1)
                    cx01 = psum.tile([P, 2 * SC], DT.float32, tag="cx", bufs=1,
                                     name=f"cx{pr}_{qc}")
                s01 = pend.pop(t)
                # ACT: exp(scores/8 + mask_k), psum -> sbuf bf16, both heads
                px = pexpp.tile([P, 2, SC], DT.bfloat16, tag="pexp",
                                name=f"px{t}")
                nc.scalar.activation(
                    out=px[:], in_=s01[:], func=AF.Exp,
                    bias=mask_sb[:, kt:kt + 1], scale=1.0 / math.sqrt(D_KV),
                )
                # DVE: multiply by exp(bias) Toeplitz, both heads in one op
                j0 = kt * P + jg0
                nc.vector.tensor_tensor(
                    px[:], px[:], u_cur[:, :, j0:j0 + SC], OP.mult
                )
                # PE: scores two iterations ahead
                if t + 2 < NTOK:
                    pend[t + 2] = emit_scores(*decode(t + 2), t + 2)
                # feeder matmuls keep the PE saturated past ACT's rate (and
                # give the px chain latency cover before PV needs it)
                pump()
                if kt in (7, 8):
                    pump()
                # PE: PV for both heads (even: [V|ones], odd: [ones|V])
                nc.tensor.matmul(
                    cx01[:, 0:SC], vaug[:, kt, 2 * pr, :], px[:, 0, :],
                    start=(kt == 0), stop=(kt == NKT - 1),
                )
                nc.tensor.matmul(
                    cx01[:, SC:2 * SC], vaug[:, kt, 2 * pr + 1, :], px[:, 1, :],
                    start=(kt == 0), stop=(kt == NKT - 1),
                )
                if kt == NKT - 1:
                    # drain slot: ctx rows + denominator rows (all DVE,
                    # partition-aligned), then deferred normalize
                    cx0, cx1 = cx01[:, 0:SC], cx01[:, SC:2 * SC]
                    # h0 ctx copy on ACT, h1 on DVE: both finish ~0.7us after
                    # PV(15) so the cx psum slot frees before the next slot's
                    # PV(0) arrives
                    dst0 = ctxt[0:64, pr, :]
                    nc.scalar.copy(_rev_ap(dst0, jg0), cx0[0:D_KV, :])
                    dst1 = ctxt[64:128, pr, :]
                    nc.vector.tensor_copy(_rev_ap(dst1, jg0), cx1[64:128, :])
                    dn = stage.tile([P, SC], DT.float32, tag="dn",
                                    name=f"dn{pr}_{qc}")
                    for r, (src_row, dn_row) in enumerate(((64, 64), (0, 0))):
                        csrc = (cx0, cx1)[r][src_row:src_row + 1, :]
                        dslc = dn[dn_row:dn_row + 1, :]
                        drev = bass.AP(
                            tensor=dslc.tensor,
                            offset=dslc.offset + (SC - 1),
                            ap=[list(dslc.ap[0]), [-1, SC]],
                        )
                        nc.vector.tensor_copy(drev, csrc)
                        nc.sync.dma_start(
                            den_dram[(2 * pr + r) * NQC + qc, :],
                            dn[dn_row:dn_row + 1, :],
                        )
                    norm_qc(pr, qc)
                    if pr == NMT - 1:
                        # out-proj rows for this (now fully normalized) q-range
                        st0 = (S - (qc + 1) * SC) // P
                        for st in range(st0, st0 + SC // P):
                            for ndp in range(2):
                                feeders.append(out_chunk_gen(st, ndp))
                    if qc == NQC - 1:
                        u_cur = u_next

            # ---------- phase C: remaining output projection ----------
            phase_c[0] = True
            while pump():
                pass

    nc.finalize()
    return nc


_NC_CACHE = None


def _get_nc():
    global _NC_CACHE
    if _NC_CACHE is None:
        _NC_CACHE = _build()
    return _NC_CACHE


def _in_maps(hidden_states, attention_mask, Wq, Wk, Wv, Wo, rel_emb):
    import ml_dtypes
    bf16 = ml_dtypes.bfloat16
    # host-side structural gather: exp(rel bias) along score diagonals
    ediag_full = np.exp(rel_emb[_BUCKETS, :].astype(np.float64)).astype(bf16)
    maps = []
    for c in range(NCORES):
        b, g = c // 4, c % 4
        hlo, hhi = g * HL, (g + 1) * HL
        def _wlayout_qk(w):  # [D, hd] -> [p, pr, kt, c] pair-major
            return np.ascontiguousarray(
                w.reshape(NDT, P, NMT, P).transpose(1, 2, 0, 3)
            ).astype(bf16)

        def _wlayout(w):  # [D, hd] -> [(p kt) h] SBUF-matched layout
            return np.ascontiguousarray(
                w.reshape(NDT, P, HL * D_KV).transpose(1, 0, 2).reshape(D, HL * D_KV)
            ).astype(bf16)
        wo_c = Wo[hlo * D_KV:hhi * D_KV, :]
        maps.append({
            "xt": np.ascontiguousarray(hidden_states[b].T).astype(bf16),
            "wq": _wlayout_qk(Wq[:, hlo * D_KV:hhi * D_KV]),
            "wk": _wlayout_qk(Wk[:, hlo * D_KV:hhi * D_KV]),
            "wv": _wlayout(Wv[:, hlo * D_KV:hhi * D_KV]),
            "wo": np.ascontiguousarray(
                wo_c.reshape(NMT, P, D).transpose(1, 0, 2).reshape(HL * D_KV, D)
            ).astype(bf16),
            "mask": np.ascontiguousarray(
                attention_mask[b, 0, 0, :].reshape(NKT, P).T.reshape(S)
            ).astype(np.float32),
            "ediag": np.ascontiguousarray(ediag_full[:, hlo:hhi].T),
        })
    return maps


def kernel(hidden_states, attention_mask, Wq, Wk, Wv, Wo, rel_emb, _trace=False,
           _trace_kwargs=None):
    hidden_states = np.asarray(hidden_states, dtype=np.float32)
    attention_mask = np.asarray(attention_mask, dtype=np.float32)
    Wq = np.asarray(Wq, dtype=np.float32)
    Wk = np.asarray(Wk, dtype=np.float32)
    Wv = np.asarray(Wv, dtype=np.float32)
    Wo = np.asarray(Wo, dtype=np.float32)
    rel_emb = np.asarray(rel_emb, dtype=np.float32)

    nc = _get_nc()
    maps = _in_maps(hidden_states, attention_mask, Wq, Wk, Wv, Wo, rel_emb)
    kw = dict(_trace_kwargs or {})
    res = run_bass_kernel_spmd(nc, maps, core_ids=list(range(NCORES)),
                               trace=_trace, **kw)
    kernel.last_results = res
    outp = np.empty((B, S, D), dtype=np.float32)
    for b in range(B):
        acc = np.asarray(res.results[4 * b]["out"], dtype=np.float32).copy()
        for g in range(1, 4):
            acc += np.asarray(res.results[4 * b + g]["out"], dtype=np.float32)
        outp[b] = acc
    return outp
